# revision 1
# baseline (speedup 1.0000x reference)
"""OIM loss with circular queue — Trainium2 Bass kernel (8 NeuronCores).

loss = mean_b [ M + log(sum_{q good} exp(30*cos(x_b,e_q) - M)) - 30*cos(x_b,e_{xe_b}) ]

where e is the circular queue after the (sequential, data-dependent) update.
The update writes U=256 normalized per-pid mean embeddings into a contiguous
window of slots and invalidates stale slots; the integer bookkeeping runs on
the host, the FLOPs on the 8 cores.

Sharding (tensor-parallel over Q; we own the shard assignment):
  * each core owns 32 of the 256 window slots (placed at columns 0..31 of its
    shard) plus up to 2016 good non-window queue slots (zero-padded);
    invalidated/bad slots are never shipped, so no masking is needed — the
    exp row-sum comes straight from the ACT accumulator.
  * the host ships emb pre-transposed (d-major) fp8-quantized in DoubleRow
    layout, and x twice: b-major bf16 (masked means) and d-major fp8
    DoubleRow tiles of the row-normalized x (per-row scaling folded into the
    fp8 quantization), so the logits matmul emits cosines directly.

Per-core device program:
  phase A (streamed, 8 batches of 4 b-tiles): masked-sum matmuls (bf16)
    accumulate in PSUM.
  finalize: normalize the 32 per-pid mean rows (norm of the raw sum — the
    count cancels; rsqrt via Newton on DVE, keeping ACT exp-only),
    PE-transpose into the window columns of the emb tile.
  phase C (streamed over 32 b-tiles): 8 fp8 DoubleRow matmuls fill a
    [128,2048] PSUM tile with cosines; one Exp activation (scale=30,
    bias=-M) writes bf16 exps whose row-sum is taken on DVE in 4x fast
    mode (cheaper than the ACT accumulator-read penalty); a small DVE
    gather extracts the target cosine from the 32 window columns.
Host: S_b = sum_c sume_c - n_zero*e^-M;  loss = mean(M + log S_b - t30_b).
"""

import os
import sys

import numpy as np

for _p in ("/opt/trn_rl_repo", "/root/.axon_site/_ro/trn_rl_repo"):
    if os.path.isdir(_p) and _p not in sys.path:
        sys.path.insert(0, _p)

B, D, Q, U = 4096, 512, 16384, 256
N_CORES = 8
UC = U // N_CORES           # 32 window slots / uniq pids per core
NW = 2016                   # non-window columns per core (zero-padded)
QSC = UC + NW               # 2048 queue columns per core
MT = B // 128               # 32 b-tiles
MB = 2                      # phase-A b-tiles per DMA batch
MC = 4                      # phase-C b-tiles per DMA batch
XG_DEFAULT = 512            # gathered rows per core for the masked means
OIM_SCALAR = 30.0
M_BIAS = 30.0               # logits are <= 30 (both sides unit-norm)
IGNORE = -1
RSQRT_MAGIC = 0x5F3759DF

_PROG_CACHE = {}

# psum/rhs column chunks — each is one full 2KB PSUM bank and ONE matmul
# accumulation group (PSUM start/stop semantics are bank-granular).
_CHUNKS = [(512, 1024), (1024, 1536), (1536, 2048), (0, 512)]


def _build_program(variant="fp8", xg_rows=XG_DEFAULT):
    import concourse.bacc as bacc
    import concourse.tile as tile
    from concourse import mybir
    from concourse.masks import make_identity

    f32 = mybir.dt.float32
    f32r = mybir.dt.float32r
    i32 = mybir.dt.int32
    bf16 = mybir.dt.bfloat16
    fp8 = mybir.dt.float8e4
    AF = mybir.ActivationFunctionType
    OP = mybir.AluOpType
    DR = mybir.MatmulPerfMode.DoubleRow

    XG, GT = xg_rows, xg_rows // 128
    fp8_mode = variant == "fp8"
    e_dt = fp8 if fp8_mode else f32        # storage dtype of x^T / emb^T
    KD = 2 if fp8_mode else 4              # matmul contraction chunks
    SUB = D // (128 * KD)                  # 2 (DoubleRow pair) or 1

    def as_mm(ap):                         # matmul-operand view
        return ap if fp8_mode else ap.bitcast(f32r)

    nc = bacc.Bacc("TRN2", target_bir_lowering=False, debug=False,
                   num_devices=N_CORES)

    xg_d = nc.dram_tensor("xg", [XG, D], bf16, kind="ExternalInput").ap()
    xt_d = nc.dram_tensor("xt", [B, D], e_dt, kind="ExternalInput").ap()
    emb_d = nc.dram_tensor("emb", [128, (D // 128) * NW], e_dt,
                           kind="ExternalInput").ap()
    meta_d = nc.dram_tensor("meta", [128, GT + UC + MT + UC], f32,
                            kind="ExternalInput").ap()
    sume_d = nc.dram_tensor("sume", [128, MT], f32, kind="ExternalOutput").ap()
    tco_d = nc.dram_tensor("tco", [128, MT], f32, kind="ExternalOutput").ap()

    with tile.TileContext(nc) as tc:
        with (
            tc.tile_pool(name="singles", bufs=1) as singles,
            tc.tile_pool(name="xwork", bufs=8) as xwork,
            tc.tile_pool(name="twork", bufs=3) as twork,
            tc.tile_pool(name="mwork", bufs=4) as mwork,
            tc.tile_pool(name="ework", bufs=4) as ework,
        ):
            # ---------- small resident inputs, one DMA ----------
            meta = singles.tile([128, GT + UC + MT + UC], f32)
            nc.sync.dma_start(out=meta, in_=meta_d)
            labg = meta[:, 0:GT]
            uniqc = meta[:, GT:GT + UC]
            widx = meta[:, GT + UC:GT + UC + MT]
            iotac = meta[:, GT + UC + MT:]

            biasM = singles.tile([128, 1], f32)
            nc.vector.memset(biasM, -M_BIAS)
            # preload the Exp activation table while DMAs stream
            junk1 = singles.tile([128, 1], f32)
            nc.scalar.activation(out=junk1, in_=biasM, func=AF.Exp)

            ident = singles.tile([128, 128], f32)
            make_identity(nc, ident)

            sacc = singles.tile([128, MT], f32)     # sum-exp out
            tsb = singles.tile([128, MT], f32)      # raw target dot out
            # queue columns: 0..31 window (written on-device), 32.. from DMA
            embt = singles.tile([128, KD, SUB, QSC], e_dt)

            # ---------- phase A: masked sums ----------
            with tc.tile_pool(name="psA", bufs=1, space="PSUM") as psA:
                ps_u = psA.tile([UC, D], f32, name="ps_u")
                for mb in range(-(-GT // MB)):
                    t0, t1 = mb * MB, min((mb + 1) * MB, GT)
                    xg4 = xwork.tile([128, t1 - t0, D], bf16, tag="xg")
                    nc.sync.dma_start(
                        out=xg4,
                        in_=xg_d[t0 * 128:t1 * 128, :]
                        .rearrange("(j p) d -> p j d", j=t1 - t0))
                    for j in range(t1 - t0):
                        m = t0 + j
                        mask = mwork.tile([128, UC], bf16, tag="mask")
                        nc.vector.tensor_scalar(out=mask, in0=uniqc,
                                                scalar1=labg[:, m:m + 1],
                                                scalar2=None, op0=OP.is_equal)
                        nc.tensor.matmul(ps_u, mask, xg4[:, j],
                                         start=(m == 0), stop=(m == GT - 1))

                # emb shard: queued after xg, needed by phase C
                nc.sync.dma_start(
                    out=embt[:, :, :, UC:],
                    in_=emb_d.rearrange("p (a b c) -> p a b c", a=KD, b=SUB))

                # ---------- finalize ----------
                # Square (ACT, shares the Exp table) first, then the DVE
                # copy runs in its shadow (ps_u readers are chained)
                scrU = singles.tile([UC, D], f32)
                ssqu = singles.tile([UC, 1], f32)
                nc.scalar.activation(out=scrU, in_=ps_u, func=AF.Square,
                                     accum_out=ssqu)
                uembS = singles.tile([UC, D], f32)
                nc.scalar.copy(out=uembS, in_=ps_u)
                # rinvu = rsqrt(ssqu) via bit-trick + 2 Newton steps (DVE-only
                # so the ACT engine never swaps away from the Exp table).
                # Clamp: ssqu=0 (all-invalid pid) would overflow y^2 -> NaN.
                nc.vector.tensor_scalar_max(out=ssqu, in0=ssqu, scalar1=1e-12)
                yi = singles.tile([UC, 1], i32)
                nc.vector.tensor_scalar(
                    out=yi, in0=ssqu.bitcast(i32), scalar1=1,
                    scalar2=None, op0=OP.arith_shift_right)
                nc.vector.tensor_scalar(
                    out=yi, in0=yi, scalar1=-1, scalar2=RSQRT_MAGIC,
                    op0=OP.mult, op1=OP.add)
                y = yi.bitcast(f32)
                t0 = singles.tile([UC, 1], f32)
                for _ in range(2):
                    nc.vector.tensor_tensor(out=t0, in0=y, in1=y, op=OP.mult)
                    nc.vector.tensor_tensor(out=t0, in0=t0, in1=ssqu,
                                            op=OP.mult)
                    nc.vector.tensor_scalar(out=t0, in0=t0, scalar1=-0.5,
                                            scalar2=1.5, op0=OP.mult,
                                            op1=OP.add)
                    nc.vector.tensor_tensor(out=y, in0=y, in1=t0, op=OP.mult)
                # normalization folds into the transpose: diag(y) as rhs
                diagY = singles.tile([UC, UC], f32)
                nc.vector.tensor_scalar_mul(out=diagY,
                                            in0=ident[0:UC, 0:UC], scalar1=y)


            # ---------- phase C: cosines + fused exp/sum + target gather ---
            # first batch is a single tile so m=0 starts as soon as possible
            batches = [(0, 1)] + [(1 + k * MC, min(1 + (k + 1) * MC, MT))
                                  for k in range((MT - 1 + MC - 1) // MC)]
            with tc.tile_pool(name="psC", bufs=2, space="PSUM") as psC:
                # scaled transposes of the mean rows -> emb window columns,
                # in a throwaway tile occupying pool slot A (m=0 then gets
                # slot B: no read-after hazard on its own tile)
                psW = psC.tile([128, QSC], f32, tag="psm")
                for k4 in range(4):
                    nc.tensor.matmul(psW[:, k4 * UC:(k4 + 1) * UC],
                                     uembS[:, k4 * 128:(k4 + 1) * 128],
                                     diagY)
                nc.vector.tensor_copy(
                    out=embt[:, :, :, 0:UC].rearrange("p a b c -> p (a b) c"),
                    in_=psW[:, 0:4 * UC].rearrange("p (k c) -> p k c", k=4))
                for (b0, b1) in batches:
                    nb = b1 - b0
                    tl4 = twork.tile([128, nb, D], e_dt, tag=f"tl{nb}")
                    nc.sync.dma_start(
                        out=tl4,
                        in_=xt_d[b0 * 128:b1 * 128, :]
                        .rearrange("(j p) d -> p j d", j=nb))
                    for j in range(nb):
                        m = b0 + j
                        tlm = tl4[:, j].rearrange("p (a b c) -> p a b c",
                                                  a=KD, b=SUB)
                        psm = psC.tile([128, QSC], f32, tag="psm")
                        pm = DR if fp8_mode else None

                        def mm(kd, p0, p1):
                            lhs = as_mm(tlm[:, kd] if fp8_mode
                                        else tlm[:, kd, 0])
                            rC = as_mm(embt[:, kd, :, p0:p1] if fp8_mode
                                       else embt[:, kd, 0, p0:p1])
                            nc.tensor.matmul(psm[:, p0:p1], lhs, rC,
                                             start=(kd == 0),
                                             stop=(kd == KD - 1),
                                             perf_mode=pm)

                        for (p0, p1) in _CHUNKS:
                            for kd in range(KD):
                                mm(kd, p0, p1)
                        g32 = mwork.tile([128, UC], f32, tag="g32")
                        nc.vector.scalar_tensor_tensor(
                            out=g32, in0=iotac, scalar=widx[:, m:m + 1],
                            in1=psm[:, 0:UC], op0=OP.is_equal, op1=OP.mult,
                            accum_out=tsb[:, m:m + 1])
                        expt = ework.tile([128, QSC], bf16, tag="expt")
                        if m == MT - 1:
                            # last tile: ACT accumulator (187ns once) beats
                            # the DVE-sum latency on the critical tail
                            nc.scalar.activation(out=expt, in_=psm,
                                                 func=AF.Exp, bias=biasM,
                                                 scale=OIM_SCALAR,
                                                 accum_out=sacc[:, m:m + 1])
                        else:
                            nc.scalar.activation(out=expt, in_=psm,
                                                 func=AF.Exp, bias=biasM,
                                                 scale=OIM_SCALAR)
                            # row-sum on DVE (4x mode on all-bf16 operands)
                            # avoids the ACT accumulator-read penalty
                            sj = ework.tile([128, QSC], bf16, tag="sj")
                            nc.vector.tensor_scalar(
                                out=sj, in0=expt, scalar1=1.0, scalar2=None,
                                op0=OP.mult, op1=OP.add,
                                accum_out=sacc[:, m:m + 1])

            nc.sync.dma_start(out=sume_d, in_=sacc)
            nc.sync.dma_start(out=tco_d, in_=tsb)

    nc.compile()
    return nc


def _host_bookkeeping(labels, label_cq, header_cq):
    """Mirror the reference's integer-only queue-update semantics."""
    labels = np.asarray(labels).astype(np.int64)
    lab = np.asarray(label_cq).astype(np.int64).copy()
    h0 = int(np.asarray(header_cq))

    uq = np.unique(labels)
    if uq.size < U:
        uniq = np.concatenate([uq, np.full(U - uq.size, uq.min(), np.int64)])
    else:
        uniq = uq[:U]

    emb_src = np.full(Q, -1, np.int64)   # >=0: window slot written by uniq u
    h = h0 % Q
    for u in range(U):
        y = uniq[u]
        m = lab == y
        i = int(np.argmax(m)) if m.any() else 0
        inval = bool(m.any()) and (i != h)
        emb_src[h] = u
        lab[h] = y
        if inval:
            lab[i] = IGNORE
        h = (h + 1) % Q

    good = lab != IGNORE
    goodidx = np.flatnonzero(good)
    gl = lab[goodidx]
    vals, first = np.unique(gl, return_index=True)
    pos = np.searchsorted(vals, labels)
    assert np.all(vals[np.clip(pos, 0, vals.size - 1)] == labels), \
        "batch label missing from queue"
    xe = goodidx[first[pos]]
    return uniq, emb_src, good, xe


def _pmajor(v, cols, dt):
    return np.ascontiguousarray(np.asarray(v, np.float64)
                                .reshape(cols, 128).T.astype(dt))


def _prepare(inputs, labels, emb_cq, label_cq, header_cq, variant):
    import ml_dtypes
    bf16 = ml_dtypes.bfloat16
    fp8_mode = variant == "fp8"
    e_dt = ml_dtypes.float8_e4m3 if fp8_mode else np.float32
    KD = 2 if fp8_mode else 4
    SUB = D // (128 * KD)

    x = np.ascontiguousarray(np.asarray(inputs, np.float32))
    emb_cq = np.ascontiguousarray(np.asarray(emb_cq, np.float32))

    uniq, emb_src, good, xe = _host_bookkeeping(labels, label_cq, header_cq)

    w_idx = emb_src[xe]                       # target window index, -1=extra
    extra = np.flatnonzero(w_idx < 0)

    # window slot of uniq u; invalidated duplicates become zero columns
    h0 = int(np.asarray(header_cq)) % Q
    wslot = (h0 + np.arange(U)) % Q
    u_valid = good[wslot]
    uniq_send = np.where(u_valid, uniq, -999).astype(np.float64)

    # d-major row-normalized fp8 x for the logits lhsT (the per-row 1/|x| is
    # folded into the quantization)
    xn = x / np.maximum(np.linalg.norm(x, axis=1, keepdims=True), 1e-12)
    Y = xn.astype(e_dt).reshape(MT, 128, KD, SUB, 128)
    xt = np.ascontiguousarray(Y.transpose(0, 4, 2, 3, 1).reshape(B, D))
    xbf = x.astype(bf16)
    labels_i = np.asarray(labels).astype(np.int64)

    # queue columns: good non-window slots split across cores
    nonwin = np.flatnonzero(good & (emb_src < 0))
    parts = np.array_split(nonwin, N_CORES)

    base = {"xt": xt}
    widx_pm = _pmajor(w_idx, MT, np.float32)
    # gathered-row count per core (multiple of 512, >= max over cores)
    gather_rows = []
    for c in range(N_CORES):
        uc_vals = uniq_send[c * UC:(c + 1) * UC]
        gather_rows.append(
            np.flatnonzero(np.isin(labels_i, uc_vals[uc_vals >= 0])))
    max_rows = max(r.size for r in gather_rows)
    XG = max(XG_DEFAULT, 128 * -(-max_rows // 128))
    GT = XG // 128
    in_maps = []
    n_pad_total = 0
    for c in range(N_CORES):
        cols = parts[c]
        n_pad_total += NW - cols.size
        E = np.zeros((NW, D), np.float32)
        E[: cols.size] = emb_cq[cols]
        Z = E.astype(e_dt).reshape(NW, KD, SUB, 128)
        embp = np.ascontiguousarray(
            Z.transpose(3, 1, 2, 0).reshape(128, KD * SUB * NW))
        # rows whose labels fall in this core's uniq set (masked-mean input)
        uc_vals = uniq_send[c * UC:(c + 1) * UC]
        rows = gather_rows[c]
        xg = np.zeros((XG, D), bf16)
        xg[: rows.size] = xbf[rows]
        labgv = np.full(XG, -1.0, np.float64)
        labgv[: rows.size] = labels_i[rows]
        meta = np.concatenate([
            _pmajor(labgv, GT, np.float32),
            np.broadcast_to(uc_vals.astype(np.float32), (128, UC)),
            widx_pm,
            np.broadcast_to(np.arange(c * UC, (c + 1) * UC,
                                      dtype=np.float32), (128, UC)),
        ], axis=1)
        in_maps.append({
            **base,
            "emb": embp,
            "xg": xg,
            "meta": np.ascontiguousarray(meta),
        })
    # zero columns (padding + invalidated window slots) each add e^-M per row
    n_const = n_pad_total + int((~u_valid).sum())
    return in_maps, extra, xe, n_const, (x, emb_cq), XG


def _combine(res_list, extra, xe, n_const, xemb):
    x, emb_cq = xemb
    S = np.zeros(B, np.float64)
    t30 = np.zeros(B, np.float64)
    for r in res_list:
        S += r["sume"].astype(np.float64).T.reshape(B)
        t30 += OIM_SCALAR * r["tco"].astype(np.float64).T.reshape(B)
    S -= n_const * np.exp(-float(M_BIAS))

    if extra.size:  # targets pointing at original (non-window) queue rows
        xb = x[extra].astype(np.float64)
        xb /= np.maximum(np.linalg.norm(xb, axis=1, keepdims=True), 1e-12)
        eb = emb_cq[xe[extra]].astype(np.float64)
        t30[extra] = OIM_SCALAR * (xb * eb).sum(axis=1)

    loss = np.mean(M_BIAS + np.log(S) - t30)
    return np.array(loss, dtype=np.float32)


def kernel(inputs, labels, emb_cq, label_cq, age_cq, header_cq):
    from concourse.bass_utils import run_bass_kernel_spmd

    variant = os.environ.get("BASS_VARIANT", "fp8")
    in_maps, extra, xe, n_const, xemb, xg_rows = _prepare(
        inputs, labels, emb_cq, label_cq, header_cq, variant)

    key = (variant, xg_rows)
    if key not in _PROG_CACHE:
        _PROG_CACHE[key] = _build_program(variant, xg_rows)
    nc = _PROG_CACHE[key]

    res = run_bass_kernel_spmd(nc, in_maps, core_ids=list(range(N_CORES)))
    return _combine(res.results, extra, xe, n_const, xemb)



# revision 2
# speedup vs baseline: 1.0196x; 1.0196x over previous
"""OIM loss with circular queue — Trainium2 Bass kernel (8 NeuronCores).

loss = mean_b [ M + log(sum_{q good} exp(30*cos(x_b,e_q) - M)) - 30*cos(x_b,e_{xe_b}) ]

where e is the circular queue after the (sequential, data-dependent) update.

Split of labor:
  host: the integer queue-update bookkeeping, the per-pid masked means
    (normalized exactly, fp8-quantized — they become ordinary queue columns),
    the target cosines t30_b (exact f64 dot with the normalized mean), and
    the final log/mean. The heavy B x Q x D cosine matmul and the B x Q
    exponentials run on the 8 cores.
  device (per core, tensor-parallel over Q): 2048 queue columns (32 window
    slots + up to 2016 good non-window slots, zero-padded). 32 b-tiles of
    fp8 DoubleRow matmuls fill a [128,2048] PSUM tile with cosines; the
    exponentials are split across three engines:
      * ACT: native Exp (scale=30, bias=-M) on cols [0:FA] with accum_out
        giving that range's row-sum directly.
      * DVE: Schraudolph exp on cols [FA:2048] — one fused mult+add
        tensor_scalar emitting int16 bf16-bit-patterns (exp(z) ~=
        bitcast_bf16(rint(z*128*log2e + 16256 + C)), C calibrated so the
        softmax-sum error is ~1e-4).
      * Pool (gpsimd): pairwise halving-add of the bf16 exps (SBUF-only
        engine), then DVE row-sums the halved row in 4x mode.
Host: S_b = sum_c (sA + sD)_c - n_zero*e^-M;  loss = mean(M + log S_b - t30_b).
"""

import os
import sys

import numpy as np

for _p in ("/opt/trn_rl_repo", "/root/.axon_site/_ro/trn_rl_repo"):
    if os.path.isdir(_p) and _p not in sys.path:
        sys.path.insert(0, _p)

B, D, Q, U = 4096, 512, 16384, 256
N_CORES = 8
UC = U // N_CORES           # 32 window slots per core
QSC = 2048                  # queue columns per core
NW = QSC - UC               # non-window columns per core (zero-padded)
MT = B // 128               # 32 b-tiles
MC = 4                      # b-tiles per xt DMA batch
KD = 2                      # matmul contraction chunks (DoubleRow pairs)
SUB = D // (128 * KD)       # 2
OIM_SCALAR = 30.0
M_BIAS = 30.0               # logits are <= 30 (both sides unit-norm)
IGNORE = -1

# ACT-exp columns; DVE schraudolphs the rest. (env-tunable for sweeps)
F_A = int(os.environ.get("BASS_FA", "1120"))
F_D = QSC - F_A
HF = F_D // 2

# Schraudolph-exp constants for exp(30*c - 30) emitted as bf16 bit patterns:
# i16 = rint(c*SCH_A + SCH_B); SCH_B holds the -30 bias, the bf16 exponent
# offset (127<<7) and the calibration constant C=-7.368 (zero weighted error
# over the cosine distribution of random unit vectors at D=512).
SCH_A = 5539.948957013619
SCH_B = 10708.683087674835

_PROG_CACHE = {}

# psum/rhs column chunks — each is one full 2KB PSUM bank and ONE matmul
# accumulation group (PSUM start/stop semantics are bank-granular). The DVE
# region [F_A:] is produced first so its longer exp chain starts early.
_CHUNKS = [(1536, 2048), (1024, 1536), (0, 512), (512, 1024)]


def _build_program(f_a=F_A):
    import concourse.bacc as bacc
    import concourse.tile as tile
    from concourse import mybir

    f32 = mybir.dt.float32
    i16 = mybir.dt.int16
    bf16 = mybir.dt.bfloat16
    fp8 = mybir.dt.float8e4
    AF = mybir.ActivationFunctionType
    OP = mybir.AluOpType
    DR = mybir.MatmulPerfMode.DoubleRow

    f_d = QSC - f_a
    hf = f_d // 2

    nc = bacc.Bacc("TRN2", target_bir_lowering=False, debug=False,
                   num_devices=N_CORES)

    xt_d = nc.dram_tensor("xt", [B, D], fp8, kind="ExternalInput").ap()
    emb_d = nc.dram_tensor("emb", [128, (D // 128) * QSC], fp8,
                           kind="ExternalInput").ap()
    sacc_d = nc.dram_tensor("sacc", [128, 2 * MT], f32,
                            kind="ExternalOutput").ap()

    with tile.TileContext(nc) as tc:
        with (
            tc.tile_pool(name="singles", bufs=1) as singles,
            tc.tile_pool(name="twork", bufs=3) as twork,
            tc.tile_pool(name="ework", bufs=2) as ework,
        ):
            biasM = singles.tile([128, 1], f32)
            nc.vector.memset(biasM, -M_BIAS)
            # preload the Exp activation table while DMAs stream
            junk1 = singles.tile([128, 1], f32)
            nc.scalar.activation(out=junk1, in_=biasM, func=AF.Exp)

            sacc = singles.tile([128, 2 * MT], f32)

            # queue columns, all from DMA (window means host-computed);
            # chunked so the first-needed columns land first
            embt = singles.tile([128, KD, SUB, QSC], fp8)
            emb4 = emb_d.rearrange("p (a b c) -> p a b c", a=KD, b=SUB)
            first_xt = twork.tile([128, 1, D], fp8, tag="tl1")
            nc.sync.dma_start(
                out=first_xt,
                in_=xt_d[0:128, :].rearrange("(j p) d -> p j d", j=1))
            for (c0, c1) in _CHUNKS:
                nc.sync.dma_start(out=embt[:, :, :, c0:c1],
                                  in_=emb4[:, :, :, c0:c1])

            batches = [(0, 1)] + [(1 + k * MC, min(1 + (k + 1) * MC, MT))
                                  for k in range((MT - 1 + MC - 1) // MC)]
            with tc.tile_pool(name="psC", bufs=2, space="PSUM") as psC:
                for (b0, b1) in batches:
                    nb = b1 - b0
                    if b0 == 0:
                        tl4 = first_xt
                    else:
                        tl4 = twork.tile([128, nb, D], fp8, tag=f"tl{nb}")
                        nc.sync.dma_start(
                            out=tl4,
                            in_=xt_d[b0 * 128:b1 * 128, :]
                            .rearrange("(j p) d -> p j d", j=nb))
                    for j in range(nb):
                        m = b0 + j
                        tlm = tl4[:, j].rearrange("p (a b c) -> p a b c",
                                                  a=KD, b=SUB)
                        psm = psC.tile([128, QSC], f32, tag="psm")
                        for (p0, p1) in _CHUNKS:
                            for kd in range(KD):
                                nc.tensor.matmul(psm[:, p0:p1], tlm[:, kd],
                                                 embt[:, kd, :, p0:p1],
                                                 start=(kd == 0),
                                                 stop=(kd == KD - 1),
                                                 perf_mode=DR)
                        # ACT: exp + row-sum of cols [0:f_a]
                        scrA = ework.tile([128, f_a], bf16, tag="scrA")
                        nc.scalar.activation(out=scrA, in_=psm[:, 0:f_a],
                                             func=AF.Exp, bias=biasM,
                                             scale=OIM_SCALAR,
                                             accum_out=sacc[:, m:m + 1])
                        # DVE: schraudolph exp of cols [f_a:] as bf16 bits
                        eDP = ework.tile([128, f_d], i16, tag="eDP")
                        nc.vector.tensor_scalar(out=eDP, in0=psm[:, f_a:],
                                                scalar1=SCH_A, scalar2=SCH_B,
                                                op0=OP.mult, op1=OP.add)
                        # Pool: halve by pairwise add; DVE: 4x row-sum
                        ebf = eDP.bitcast(bf16)
                        ph = ework.tile([128, hf], bf16, tag="ph")
                        nc.gpsimd.tensor_tensor(out=ph, in0=ebf[:, 0:hf],
                                                in1=ebf[:, hf:], op=OP.add)
                        sj = ework.tile([128, hf], bf16, tag="sj")
                        nc.vector.tensor_scalar(
                            out=sj, in0=ph, scalar1=1.0, scalar2=None,
                            op0=OP.mult, op1=OP.add,
                            accum_out=sacc[:, MT + m:MT + m + 1])

            nc.sync.dma_start(out=sacc_d, in_=sacc)

    nc.compile()
    return nc


def _host_bookkeeping(labels, label_cq, header_cq):
    """Mirror the reference's integer-only queue-update semantics."""
    labels = np.asarray(labels).astype(np.int64)
    lab = np.asarray(label_cq).astype(np.int64).copy()
    h0 = int(np.asarray(header_cq))

    uq = np.unique(labels)
    if uq.size < U:
        uniq = np.concatenate([uq, np.full(U - uq.size, uq.min(), np.int64)])
    else:
        uniq = uq[:U]

    emb_src = np.full(Q, -1, np.int64)   # >=0: window slot written by uniq u
    h = h0 % Q
    for u in range(U):
        y = uniq[u]
        m = lab == y
        i = int(np.argmax(m)) if m.any() else 0
        inval = bool(m.any()) and (i != h)
        emb_src[h] = u
        lab[h] = y
        if inval:
            lab[i] = IGNORE
        h = (h + 1) % Q

    good = lab != IGNORE
    goodidx = np.flatnonzero(good)
    gl = lab[goodidx]
    vals, first = np.unique(gl, return_index=True)
    pos = np.searchsorted(vals, labels)
    assert np.all(vals[np.clip(pos, 0, vals.size - 1)] == labels), \
        "batch label missing from queue"
    xe = goodidx[first[pos]]
    return uniq, emb_src, good, xe


def _prepare(inputs, labels, emb_cq, label_cq, header_cq):
    import ml_dtypes
    e_dt = ml_dtypes.float8_e4m3

    x = np.ascontiguousarray(np.asarray(inputs, np.float32))
    emb_cq = np.ascontiguousarray(np.asarray(emb_cq, np.float32))
    labels_i = np.asarray(labels).astype(np.int64)

    uniq, emb_src, good, xe = _host_bookkeeping(labels, label_cq, header_cq)

    # per-pid means over the batch (sorted-group reduceat), normalized exactly
    order = np.argsort(labels_i, kind="stable")
    ls = labels_i[order]
    starts = np.flatnonzero(np.r_[True, ls[1:] != ls[:-1]])
    vals = ls[starts]
    sums = np.add.reduceat(x[order].astype(np.float64), starts, axis=0)
    counts = np.diff(np.r_[starts, ls.size])[:, None]
    means = sums / counts
    means /= np.maximum(np.linalg.norm(means, axis=1, keepdims=True), 1e-12)

    # window columns in slot order; invalidated window slots become zeros
    h0 = int(np.asarray(header_cq)) % Q
    wslot = (h0 + np.arange(U)) % Q
    u_valid = good[wslot]
    widx_of_uniq = np.searchsorted(vals, uniq)
    win_emb = means[widx_of_uniq] * u_valid[:, None]          # [U, D] f64

    # exact target cosines on the host
    xn64 = x.astype(np.float64)
    xn64 /= np.maximum(np.linalg.norm(xn64, axis=1, keepdims=True), 1e-12)
    w_idx = emb_src[xe]                      # target window index, -1=extra
    tgt = np.empty((B, D), np.float64)
    winrows = w_idx >= 0
    tgt[winrows] = means[widx_of_uniq[np.clip(w_idx, 0, U - 1)][winrows]]
    if (~winrows).any():
        eb = emb_cq[xe[~winrows]].astype(np.float64)
        tgt[~winrows] = eb
    t30 = OIM_SCALAR * np.einsum("bd,bd->b", xn64, tgt)

    # d-major row-normalized fp8 x for the logits lhsT (per-row 1/|x| folded
    # into the quantization)
    xn = (xn64.astype(np.float32)).astype(e_dt)
    Y = xn.reshape(MT, 128, KD, SUB, 128)
    xt = np.ascontiguousarray(Y.transpose(0, 4, 2, 3, 1).reshape(B, D))

    # queue columns per core: 32 window means + good non-window slots
    nonwin = np.flatnonzero(good & (emb_src < 0))
    parts = np.array_split(nonwin, N_CORES)
    in_maps = []
    n_zero = int((~u_valid).sum())
    for c in range(N_CORES):
        cols = parts[c]
        n_zero += NW - cols.size
        E = np.zeros((QSC, D), np.float32)
        E[:UC] = win_emb[c * UC:(c + 1) * UC]
        E[UC:UC + cols.size] = emb_cq[cols]
        Z = E.astype(e_dt).reshape(QSC, KD, SUB, 128)
        embp = np.ascontiguousarray(
            Z.transpose(3, 1, 2, 0).reshape(128, KD * SUB * QSC))
        in_maps.append({"xt": xt, "emb": embp})
    return in_maps, t30, n_zero


def _combine(res_list, t30, n_zero):
    S = np.zeros(B, np.float64)
    for r in res_list:
        sa = r["sacc"].astype(np.float64)
        S += sa[:, :MT].T.reshape(B) + sa[:, MT:].T.reshape(B)
    S -= n_zero * np.exp(-float(M_BIAS))
    loss = np.mean(M_BIAS + np.log(S) - t30)
    return np.array(loss, dtype=np.float32)


def kernel(inputs, labels, emb_cq, label_cq, age_cq, header_cq):
    from concourse.bass_utils import run_bass_kernel_spmd

    in_maps, t30, n_zero = _prepare(inputs, labels, emb_cq, label_cq,
                                    header_cq)

    key = F_A
    if key not in _PROG_CACHE:
        _PROG_CACHE[key] = _build_program(key)
    nc = _PROG_CACHE[key]

    res = run_bass_kernel_spmd(nc, in_maps, core_ids=list(range(N_CORES)))
    return _combine(res.results, t30, n_zero)


# revision 5
# speedup vs baseline: 1.0387x; 1.0187x over previous
"""OIM loss with circular queue — Trainium2 Bass kernel (8 NeuronCores).

loss = mean_b [ M + log(sum_{q good} exp(30*cos(x_b,e_q) - M)) - 30*cos(x_b,e_{xe_b}) ]

where e is the circular queue after the (sequential, data-dependent) update.

Split of labor:
  host: the integer queue-update bookkeeping, the per-pid masked means
    (normalized exactly, fp8-quantized — they become ordinary queue columns),
    the target cosines t30_b (exact f64 dot with the normalized mean), and
    the final log/mean. The heavy B x Q x D cosine matmul and the B x Q
    exponentials run on the 8 cores.
  device (per core, tensor-parallel over Q): 2048 queue columns (32 window
    slots + up to 2016 good non-window slots, zero-padded). 32 b-tiles of
    fp8 DoubleRow matmuls fill a [128,2048] PSUM tile with cosines; the
    exponentials are split across three engines:
      * ACT: native Exp (scale=30, bias=-M) on cols [0:FA] with accum_out
        giving that range's row-sum directly.
      * DVE: Schraudolph exp on cols [FA:2048] — one fused mult+add
        tensor_scalar emitting int16 bf16-bit-patterns (exp(z) ~=
        bitcast_bf16(rint(z*128*log2e + 16256 + C)), C calibrated so the
        softmax-sum error is ~1e-4).
      * Pool (gpsimd): pairwise halving-add of the bf16 exps (SBUF-only
        engine), then DVE row-sums the halved row in 4x mode.
Host: S_b = sum_c (sA + sD)_c - n_zero*e^-M;  loss = mean(M + log S_b - t30_b).
"""

import os
import sys

import numpy as np

for _p in ("/opt/trn_rl_repo", "/root/.axon_site/_ro/trn_rl_repo"):
    if os.path.isdir(_p) and _p not in sys.path:
        sys.path.insert(0, _p)

B, D, Q, U = 4096, 512, 16384, 256
N_CORES = 8
UC = U // N_CORES           # 32 window slots per core
QSC = 2048                  # queue columns per core
NW = QSC - UC               # non-window columns per core (zero-padded)
MT = B // 128               # 32 b-tiles
MC = 4                      # b-tiles per xt DMA batch
KD = 2                      # matmul contraction chunks (DoubleRow pairs)
SUB = D // (128 * KD)       # 2
OIM_SCALAR = 30.0
M_BIAS = 30.0               # logits are <= 30 (both sides unit-norm)
IGNORE = -1

# ACT-exp columns; DVE schraudolphs the rest. (env-tunable for sweeps)
F_A = int(os.environ.get("BASS_FA", "1120"))
F_D = QSC - F_A
HF = F_D // 2

# Schraudolph-exp constants for exp(30*c - 30) emitted as bf16 bit patterns:
# i16 = rint(c*SCH_A + SCH_B); SCH_B holds the -30 bias, the bf16 exponent
# offset (127<<7) and the calibration constant C=-7.368 (zero weighted error
# over the cosine distribution of random unit vectors at D=512).
SCH_A = 5539.948957013619
SCH_B = 10708.683087674835

_PROG_CACHE = {}

# psum/rhs column chunks — each is one full 2KB PSUM bank and ONE matmul
# accumulation group (PSUM start/stop semantics are bank-granular). The DVE
# region [F_A:] is produced first so its longer exp chain starts early.
_CHUNKS = [(1536, 2048), (1024, 1536), (0, 512), (512, 1024)]


def _build_program(f_a=F_A):
    import concourse.bacc as bacc
    import concourse.tile as tile
    from concourse import mybir

    f32 = mybir.dt.float32
    i16 = mybir.dt.int16
    bf16 = mybir.dt.bfloat16
    fp8 = mybir.dt.float8e4
    AF = mybir.ActivationFunctionType
    OP = mybir.AluOpType
    DR = mybir.MatmulPerfMode.DoubleRow

    f_d = QSC - f_a
    hf = f_d // 2

    nc = bacc.Bacc("TRN2", target_bir_lowering=False, debug=False,
                   num_devices=N_CORES)

    xt_d = nc.dram_tensor("xt", [B, D], fp8, kind="ExternalInput").ap()
    emb_d = nc.dram_tensor("emb", [128, (D // 128) * QSC], fp8,
                           kind="ExternalInput").ap()
    sacc_d = nc.dram_tensor("sacc", [128, 2 * MT], f32,
                            kind="ExternalOutput").ap()

    with tile.TileContext(nc) as tc:
        with (
            tc.tile_pool(name="singles", bufs=1) as singles,
            tc.tile_pool(name="twork", bufs=3) as twork,
            tc.tile_pool(name="ework", bufs=3) as ework,
        ):
            biasM = singles.tile([128, 1], f32)
            nc.vector.memset(biasM, -M_BIAS)
            # preload the Exp activation table while DMAs stream
            junk1 = singles.tile([128, 1], f32)
            nc.scalar.activation(out=junk1, in_=biasM, func=AF.Exp)

            sacc = singles.tile([128, 2 * MT], f32)

            # queue columns, all from DMA (window means host-computed);
            # chunked so the first-needed columns land first
            embt = singles.tile([128, KD, SUB, QSC], fp8)
            emb4 = emb_d.rearrange("p (a b c) -> p a b c", a=KD, b=SUB)
            first_xt = twork.tile([128, 1, D], fp8, tag="tl1")
            nc.sync.dma_start(
                out=first_xt,
                in_=xt_d[0:128, :].rearrange("(j p) d -> p j d", j=1))
            for (c0, c1) in _CHUNKS:
                nc.sync.dma_start(out=embt[:, :, :, c0:c1],
                                  in_=emb4[:, :, :, c0:c1])

            batches = [(0, 1)] + [(1 + k * MC, min(1 + (k + 1) * MC, MT))
                                  for k in range((MT - 1 + MC - 1) // MC)]
            pend = []           # (m, ph) halved rows awaiting the DVE sum

            def emit_sum(nc_, sacc_, item):
                m_, ph_ = item
                sj = ework.tile([128, hf], bf16, tag="sj")
                nc_.vector.tensor_scalar(
                    out=sj, in0=ph_, scalar1=1.0, scalar2=None,
                    op0=OP.mult, op1=OP.add,
                    accum_out=sacc_[:, MT + m_:MT + m_ + 1])

            with tc.tile_pool(name="psC", bufs=2, space="PSUM") as psC:
                for (b0, b1) in batches:
                    nb = b1 - b0
                    if b0 == 0:
                        tl4 = first_xt
                    else:
                        tl4 = twork.tile([128, nb, D], fp8, tag=f"tl{nb}")
                        nc.sync.dma_start(
                            out=tl4,
                            in_=xt_d[b0 * 128:b1 * 128, :]
                            .rearrange("(j p) d -> p j d", j=nb))
                    for j in range(nb):
                        m = b0 + j
                        tlm = tl4[:, j].rearrange("p (a b c) -> p a b c",
                                                  a=KD, b=SUB)
                        psm = psC.tile([128, QSC], f32, tag="psm")
                        for (p0, p1) in _CHUNKS:
                            for kd in range(KD):
                                nc.tensor.matmul(psm[:, p0:p1], tlm[:, kd],
                                                 embt[:, kd, :, p0:p1],
                                                 start=(kd == 0),
                                                 stop=(kd == KD - 1),
                                                 perf_mode=DR)
                        # DVE: schraudolph exp of cols [f_a:] as bf16 bits
                        eDP = ework.tile([128, f_d], i16, tag="eDP")
                        nc.vector.tensor_scalar(out=eDP, in0=psm[:, f_a:],
                                                scalar1=SCH_A, scalar2=SCH_B,
                                                op0=OP.mult, op1=OP.add)
                        # ACT: exp + row-sum of cols [0:f_a]
                        scrA = ework.tile([128, f_a], bf16, tag="scrA")
                        nc.scalar.activation(out=scrA, in_=psm[:, 0:f_a],
                                             func=AF.Exp, bias=biasM,
                                             scale=OIM_SCALAR,
                                             accum_out=sacc[:, m:m + 1])
                        # Pool: halve by pairwise add ((in0*1) + in1)
                        ebf = eDP.bitcast(bf16)
                        ph = ework.tile([128, hf], bf16, tag="ph")
                        nc.gpsimd.tensor_tensor(out=ph, in0=ebf[:, 0:hf],
                                                in1=ebf[:, hf:], op=OP.add)
                        # DVE 4x row-sum of tile m-1's halved row (delayed one
                        # tile so the Pool round-trip overlaps the next
                        # schraudolph instead of stalling the DVE stream)
                        pend.append((m, ph))
                        if len(pend) > 1:
                            emit_sum(nc, sacc, pend.pop(0))
                while pend:
                    emit_sum(nc, sacc, pend.pop(0))

            nc.sync.dma_start(out=sacc_d, in_=sacc)

    nc.compile()
    return nc


def _host_bookkeeping(labels, label_cq, header_cq):
    """Mirror the reference's integer-only queue-update semantics."""
    labels = np.asarray(labels).astype(np.int64)
    lab = np.asarray(label_cq).astype(np.int64).copy()
    h0 = int(np.asarray(header_cq))

    uq = np.unique(labels)
    if uq.size < U:
        uniq = np.concatenate([uq, np.full(U - uq.size, uq.min(), np.int64)])
    else:
        uniq = uq[:U]

    emb_src = np.full(Q, -1, np.int64)   # >=0: window slot written by uniq u
    h = h0 % Q
    for u in range(U):
        y = uniq[u]
        m = lab == y
        i = int(np.argmax(m)) if m.any() else 0
        inval = bool(m.any()) and (i != h)
        emb_src[h] = u
        lab[h] = y
        if inval:
            lab[i] = IGNORE
        h = (h + 1) % Q

    good = lab != IGNORE
    goodidx = np.flatnonzero(good)
    gl = lab[goodidx]
    vals, first = np.unique(gl, return_index=True)
    pos = np.searchsorted(vals, labels)
    assert np.all(vals[np.clip(pos, 0, vals.size - 1)] == labels), \
        "batch label missing from queue"
    xe = goodidx[first[pos]]
    return uniq, emb_src, good, xe


def _prepare(inputs, labels, emb_cq, label_cq, header_cq):
    import ml_dtypes
    e_dt = ml_dtypes.float8_e4m3

    x = np.ascontiguousarray(np.asarray(inputs, np.float32))
    emb_cq = np.ascontiguousarray(np.asarray(emb_cq, np.float32))
    labels_i = np.asarray(labels).astype(np.int64)

    uniq, emb_src, good, xe = _host_bookkeeping(labels, label_cq, header_cq)

    # per-pid means over the batch (sorted-group reduceat), normalized exactly
    order = np.argsort(labels_i, kind="stable")
    ls = labels_i[order]
    starts = np.flatnonzero(np.r_[True, ls[1:] != ls[:-1]])
    vals = ls[starts]
    sums = np.add.reduceat(x[order].astype(np.float64), starts, axis=0)
    counts = np.diff(np.r_[starts, ls.size])[:, None]
    means = sums / counts
    means /= np.maximum(np.linalg.norm(means, axis=1, keepdims=True), 1e-12)

    # window columns in slot order; invalidated window slots become zeros
    h0 = int(np.asarray(header_cq)) % Q
    wslot = (h0 + np.arange(U)) % Q
    u_valid = good[wslot]
    widx_of_uniq = np.searchsorted(vals, uniq)
    win_emb = means[widx_of_uniq] * u_valid[:, None]          # [U, D] f64

    # exact target cosines on the host
    xn64 = x.astype(np.float64)
    xn64 /= np.maximum(np.linalg.norm(xn64, axis=1, keepdims=True), 1e-12)
    w_idx = emb_src[xe]                      # target window index, -1=extra
    tgt = np.empty((B, D), np.float64)
    winrows = w_idx >= 0
    tgt[winrows] = means[widx_of_uniq[np.clip(w_idx, 0, U - 1)][winrows]]
    if (~winrows).any():
        eb = emb_cq[xe[~winrows]].astype(np.float64)
        tgt[~winrows] = eb
    t30 = OIM_SCALAR * np.einsum("bd,bd->b", xn64, tgt)

    # d-major row-normalized fp8 x for the logits lhsT (per-row 1/|x| folded
    # into the quantization)
    xn = (xn64.astype(np.float32)).astype(e_dt)
    Y = xn.reshape(MT, 128, KD, SUB, 128)
    xt = np.ascontiguousarray(Y.transpose(0, 4, 2, 3, 1).reshape(B, D))

    # queue columns per core: 32 window means + good non-window slots
    nonwin = np.flatnonzero(good & (emb_src < 0))
    parts = np.array_split(nonwin, N_CORES)
    in_maps = []
    n_zero = int((~u_valid).sum())
    for c in range(N_CORES):
        cols = parts[c]
        n_zero += NW - cols.size
        E = np.zeros((QSC, D), np.float32)
        E[:UC] = win_emb[c * UC:(c + 1) * UC]
        E[UC:UC + cols.size] = emb_cq[cols]
        Z = E.astype(e_dt).reshape(QSC, KD, SUB, 128)
        embp = np.ascontiguousarray(
            Z.transpose(3, 1, 2, 0).reshape(128, KD * SUB * QSC))
        in_maps.append({"xt": xt, "emb": embp})
    return in_maps, t30, n_zero


def _combine(res_list, t30, n_zero):
    S = np.zeros(B, np.float64)
    for r in res_list:
        sa = r["sacc"].astype(np.float64)
        S += sa[:, :MT].T.reshape(B) + sa[:, MT:].T.reshape(B)
    S -= n_zero * np.exp(-float(M_BIAS))
    loss = np.mean(M_BIAS + np.log(S) - t30)
    return np.array(loss, dtype=np.float32)


def kernel(inputs, labels, emb_cq, label_cq, age_cq, header_cq):
    from concourse.bass_utils import run_bass_kernel_spmd

    in_maps, t30, n_zero = _prepare(inputs, labels, emb_cq, label_cq,
                                    header_cq)

    key = F_A
    if key not in _PROG_CACHE:
        _PROG_CACHE[key] = _build_program(key)
    nc = _PROG_CACHE[key]

    res = run_bass_kernel_spmd(nc, in_maps, core_ids=list(range(N_CORES)))
    return _combine(res.results, t30, n_zero)


# revision 6
# speedup vs baseline: 1.3500x; 1.2997x over previous
"""OIM loss with circular queue — Trainium2 Bass kernel (8 NeuronCores).

loss = mean_b [ M + log(sum_{q good} exp(30*cos(x_b,e_q) - M)) - 30*cos(x_b,e_{xe_b}) ]

where e is the circular queue after the (sequential, data-dependent) update.

Split of labor:
  host: the integer queue-update bookkeeping, the per-pid masked means
    (normalized exactly, fp8-quantized — they become ordinary queue columns),
    the target cosines t30_b (exact f64 dot with the normalized mean), and
    the final log/mean. The heavy B x Q x D cosine matmul and the B x Q
    exponentials run on the 8 cores.
  device (per core, tensor-parallel over Q): 2016 queue columns (32 window
    slots + up to 1984 good non-window slots, zero-padded). 32 b-tiles of
    fp8 DoubleRow matmuls fill two PSUM tiles (psmA [128,1024] + psmD
    [128,992] — separate tiles so their readers aren't serialized by the
    tile framework's same-tile reader chaining) with cosines; the
    exponentials are split across three engines:
      * ACT: native Exp (scale=30, bias=-M) on psmA with accum_out giving
        that range's row-sum directly.
      * DVE: Schraudolph exp on psmD — one fused mult+add tensor_scalar
        emitting int16 bf16-bit-patterns (exp(z) ~= bitcast_bf16(
        rint(z*128*log2e + 16256 + C)), C calibrated so the softmax-sum
        error is ~1e-4).
      * Pool (gpsimd): pairwise halving-add of the bf16 exps (SBUF-only
        engine), then DVE row-sums the halved row in 4x mode one tile
        later (so the Pool round-trip never stalls the DVE stream).
Host: S_b = sum_c (sA + sD)_c - n_zero*e^-M;  loss = mean(M + log S_b - t30_b).
"""

import os
import sys

import numpy as np

for _p in ("/opt/trn_rl_repo", "/root/.axon_site/_ro/trn_rl_repo"):
    if os.path.isdir(_p) and _p not in sys.path:
        sys.path.insert(0, _p)

B, D, Q, U = 4096, 512, 16384, 256
N_CORES = 8
UC = U // N_CORES           # 32 window slots per core
F_A = 1024                  # ACT exp cols (psmA, bank-aligned)
F_D = 992                   # DVE schraudolph cols (psmD)
QSC = F_A + F_D             # 2016 queue columns per core
NW = QSC - UC               # non-window columns per core (zero-padded)
HF = F_D // 2
MT = B // 128               # 32 b-tiles
MC = 4                      # b-tiles per xt DMA batch
KD = 2                      # matmul contraction chunks (DoubleRow pairs)
SUB = D // (128 * KD)       # 2
OIM_SCALAR = 30.0
M_BIAS = 30.0               # logits are <= 30 (both sides unit-norm)
IGNORE = -1
NPOOL_TAIL = 2              # last tiles skip Pool (shorter drain chain)

# Schraudolph-exp constants for exp(30*c - 30) emitted as bf16 bit patterns:
# i16 = rint(c*SCH_A + SCH_B); SCH_B holds the -30 bias, the bf16 exponent
# offset (127<<7) and the calibration constant C=-7.368 (zero weighted error
# over the cosine distribution of random unit vectors at D=512).
SCH_A = 5539.948957013619
SCH_B = 10708.683087674835

_PROG_CACHE = {}

# psum/rhs column chunks: (tile, col0, col1) — each is one PSUM bank and one
# matmul accumulation group. psmD first so the DVE exp chain starts early.
_CHUNKS = [("D", 0, 512), ("D", 512, F_D), ("A", 0, 512), ("A", 512, F_A)]


def _build_program():
    import concourse.bacc as bacc
    import concourse.tile as tile
    from concourse import mybir

    f32 = mybir.dt.float32
    i16 = mybir.dt.int16
    bf16 = mybir.dt.bfloat16
    fp8 = mybir.dt.float8e4
    AF = mybir.ActivationFunctionType
    OP = mybir.AluOpType
    DR = mybir.MatmulPerfMode.DoubleRow

    nc = bacc.Bacc("TRN2", target_bir_lowering=False, debug=False,
                   num_devices=N_CORES)

    xt_d = nc.dram_tensor("xt", [B, D], fp8, kind="ExternalInput").ap()
    emb_d = nc.dram_tensor("emb", [128, (D // 128) * QSC], fp8,
                           kind="ExternalInput").ap()
    sacc_d = nc.dram_tensor("sacc", [128, 2 * MT], f32,
                            kind="ExternalOutput").ap()

    with tile.TileContext(nc) as tc:
        with (
            tc.tile_pool(name="singles", bufs=1) as singles,
            tc.tile_pool(name="twork", bufs=3) as twork,
            tc.tile_pool(name="ework", bufs=3) as ework,
        ):
            biasM = singles.tile([128, 1], f32)
            nc.vector.memset(biasM, -M_BIAS)
            # preload the Exp activation table while DMAs stream
            junk1 = singles.tile([128, 1], f32)
            nc.scalar.activation(out=junk1, in_=biasM, func=AF.Exp)

            sacc = singles.tile([128, 2 * MT], f32)

            # queue columns, all from DMA (window means host-computed);
            # ordered so the first-needed columns land first
            embt = singles.tile([128, KD, SUB, QSC], fp8)
            emb4 = emb_d.rearrange("p (a b c) -> p a b c", a=KD, b=SUB)
            first_xt = twork.tile([128, 1, D], fp8, tag="tl1")
            nc.sync.dma_start(
                out=first_xt,
                in_=xt_d[0:128, :].rearrange("(j p) d -> p j d", j=1))
            for (t, c0, c1) in _CHUNKS:
                o = 0 if t == "D" else F_D
                nc.sync.dma_start(out=embt[:, :, :, o + c0:o + c1],
                                  in_=emb4[:, :, :, o + c0:o + c1])

            batches = [(0, 1)] + [(1 + k * MC, min(1 + (k + 1) * MC, MT))
                                  for k in range((MT - 1 + MC - 1) // MC)]
            pend = []           # (m, tile, ncols) rows awaiting the DVE sum

            def emit_sum(item):
                m_, t_, n_ = item
                sj = ework.tile([128, n_], bf16, tag="sj")
                nc.vector.tensor_scalar(
                    out=sj, in0=t_, scalar1=1.0, scalar2=None,
                    op0=OP.mult, op1=OP.add,
                    accum_out=sacc[:, MT + m_:MT + m_ + 1])

            with tc.tile_pool(name="psC", bufs=2, space="PSUM") as psC:
                for (b0, b1) in batches:
                    nb = b1 - b0
                    if b0 == 0:
                        tl4 = first_xt
                    else:
                        tl4 = twork.tile([128, nb, D], fp8, tag=f"tl{nb}")
                        nc.sync.dma_start(
                            out=tl4,
                            in_=xt_d[b0 * 128:b1 * 128, :]
                            .rearrange("(j p) d -> p j d", j=nb))
                    for j in range(nb):
                        m = b0 + j
                        tlm = tl4[:, j].rearrange("p (a b c) -> p a b c",
                                                  a=KD, b=SUB)
                        psmA = psC.tile([128, F_A], f32, tag="psmA")
                        psmD = psC.tile([128, F_D], f32, tag="psmD")
                        for (t, p0, p1) in _CHUNKS:
                            ps, o = (psmD, 0) if t == "D" else (psmA, F_D)
                            for kd in range(KD):
                                nc.tensor.matmul(ps[:, p0:p1], tlm[:, kd],
                                                 embt[:, kd, :,
                                                      o + p0:o + p1],
                                                 start=(kd == 0),
                                                 stop=(kd == KD - 1),
                                                 perf_mode=DR)
                        # DVE: schraudolph exp of psmD as bf16 bit patterns
                        eDP = ework.tile([128, F_D], i16, tag="eDP")
                        nc.vector.tensor_scalar(out=eDP, in0=psmD,
                                                scalar1=SCH_A, scalar2=SCH_B,
                                                op0=OP.mult, op1=OP.add)
                        # ACT: exp + row-sum of psmA
                        scrA = ework.tile([128, F_A], bf16, tag="scrA")
                        nc.scalar.activation(out=scrA, in_=psmA,
                                             func=AF.Exp, bias=biasM,
                                             scale=OIM_SCALAR,
                                             accum_out=sacc[:, m:m + 1])
                        ebf = eDP.bitcast(bf16)
                        if m < MT - NPOOL_TAIL:
                            # Pool: halve by pairwise add; DVE sums halved
                            # row one tile later (Pool latency overlaps the
                            # next schraudolph instead of stalling DVE)
                            ph = ework.tile([128, HF], bf16, tag="ph")
                            nc.gpsimd.tensor_tensor(out=ph, in0=ebf[:, 0:HF],
                                                    in1=ebf[:, HF:],
                                                    op=OP.add)
                            pend.append((m, ph, HF))
                        else:
                            # drain tail: skip the Pool round-trip
                            pend.append((m, ebf, F_D))
                        if len(pend) > 1:
                            emit_sum(pend.pop(0))
                while pend:
                    emit_sum(pend.pop(0))

            nc.sync.dma_start(out=sacc_d, in_=sacc)

    nc.compile()
    return nc


def _host_bookkeeping(labels, label_cq, header_cq):
    """Mirror the reference's integer-only queue-update semantics."""
    labels = np.asarray(labels).astype(np.int64)
    lab = np.asarray(label_cq).astype(np.int64).copy()
    h0 = int(np.asarray(header_cq))

    uq = np.unique(labels)
    if uq.size < U:
        uniq = np.concatenate([uq, np.full(U - uq.size, uq.min(), np.int64)])
    else:
        uniq = uq[:U]

    emb_src = np.full(Q, -1, np.int64)   # >=0: window slot written by uniq u
    h = h0 % Q
    for u in range(U):
        y = uniq[u]
        m = lab == y
        i = int(np.argmax(m)) if m.any() else 0
        inval = bool(m.any()) and (i != h)
        emb_src[h] = u
        lab[h] = y
        if inval:
            lab[i] = IGNORE
        h = (h + 1) % Q

    good = lab != IGNORE
    goodidx = np.flatnonzero(good)
    gl = lab[goodidx]
    vals, first = np.unique(gl, return_index=True)
    pos = np.searchsorted(vals, labels)
    assert np.all(vals[np.clip(pos, 0, vals.size - 1)] == labels), \
        "batch label missing from queue"
    xe = goodidx[first[pos]]
    return uniq, emb_src, good, xe


def _prepare(inputs, labels, emb_cq, label_cq, header_cq):
    import ml_dtypes
    e_dt = ml_dtypes.float8_e4m3

    x = np.ascontiguousarray(np.asarray(inputs, np.float32))
    emb_cq = np.ascontiguousarray(np.asarray(emb_cq, np.float32))
    labels_i = np.asarray(labels).astype(np.int64)

    uniq, emb_src, good, xe = _host_bookkeeping(labels, label_cq, header_cq)

    # per-pid means over the batch (sorted-group reduceat), normalized exactly
    order = np.argsort(labels_i, kind="stable")
    ls = labels_i[order]
    starts = np.flatnonzero(np.r_[True, ls[1:] != ls[:-1]])
    vals = ls[starts]
    sums = np.add.reduceat(x[order].astype(np.float64), starts, axis=0)
    counts = np.diff(np.r_[starts, ls.size])[:, None]
    means = sums / counts
    means /= np.maximum(np.linalg.norm(means, axis=1, keepdims=True), 1e-12)

    # window columns in slot order; invalidated window slots become zeros
    h0 = int(np.asarray(header_cq)) % Q
    wslot = (h0 + np.arange(U)) % Q
    u_valid = good[wslot]
    widx_of_uniq = np.searchsorted(vals, uniq)
    win_emb = means[widx_of_uniq] * u_valid[:, None]          # [U, D] f64

    # exact target cosines on the host
    xn64 = x.astype(np.float64)
    xn64 /= np.maximum(np.linalg.norm(xn64, axis=1, keepdims=True), 1e-12)
    w_idx = emb_src[xe]                      # target window index, -1=extra
    tgt = np.empty((B, D), np.float64)
    winrows = w_idx >= 0
    tgt[winrows] = means[widx_of_uniq[np.clip(w_idx, 0, U - 1)][winrows]]
    if (~winrows).any():
        eb = emb_cq[xe[~winrows]].astype(np.float64)
        tgt[~winrows] = eb
    t30 = OIM_SCALAR * np.einsum("bd,bd->b", xn64, tgt)

    # d-major row-normalized fp8 x for the logits lhsT (per-row 1/|x| folded
    # into the quantization)
    xn = (xn64.astype(np.float32)).astype(e_dt)
    Y = xn.reshape(MT, 128, KD, SUB, 128)
    xt = np.ascontiguousarray(Y.transpose(0, 4, 2, 3, 1).reshape(B, D))

    # queue columns per core: 32 window means + good non-window slots.
    # device layout: cols [0:F_D] = psmD range, [F_D:QSC] = psmA range;
    # the window means go at the start of the psmA range (arbitrary).
    nonwin = np.flatnonzero(good & (emb_src < 0))
    assert nonwin.size <= N_CORES * NW, "queue overflow vs compiled shape"
    parts = np.array_split(nonwin, N_CORES)
    in_maps = []
    n_zero = int((~u_valid).sum())
    for c in range(N_CORES):
        cols = parts[c]
        n_zero += NW - cols.size
        E = np.zeros((QSC, D), np.float32)
        E[:F_D] = emb_cq[cols[:F_D]]
        E[F_D:F_D + UC] = win_emb[c * UC:(c + 1) * UC]
        rest = cols[F_D:]
        E[F_D + UC:F_D + UC + rest.size] = emb_cq[rest]
        Z = E.astype(e_dt).reshape(QSC, KD, SUB, 128)
        embp = np.ascontiguousarray(
            Z.transpose(3, 1, 2, 0).reshape(128, KD * SUB * QSC))
        in_maps.append({"xt": xt, "emb": embp})
    return in_maps, t30, n_zero


def _combine(res_list, t30, n_zero):
    S = np.zeros(B, np.float64)
    for r in res_list:
        sa = r["sacc"].astype(np.float64)
        S += sa[:, :MT].T.reshape(B) + sa[:, MT:].T.reshape(B)
    S -= n_zero * np.exp(-float(M_BIAS))
    loss = np.mean(M_BIAS + np.log(S) - t30)
    return np.array(loss, dtype=np.float32)


def kernel(inputs, labels, emb_cq, label_cq, age_cq, header_cq):
    from concourse.bass_utils import run_bass_kernel_spmd

    in_maps, t30, n_zero = _prepare(inputs, labels, emb_cq, label_cq,
                                    header_cq)

    if "prog" not in _PROG_CACHE:
        _PROG_CACHE["prog"] = _build_program()
    nc = _PROG_CACHE["prog"]

    res = run_bass_kernel_spmd(nc, in_maps, core_ids=list(range(N_CORES)))
    return _combine(res.results, t30, n_zero)


# revision 8
# speedup vs baseline: 1.3512x; 1.0009x over previous
"""OIM loss with circular queue — Trainium2 Bass kernel (8 NeuronCores).

loss = mean_b [ M + log(sum_{q good} exp(30*cos(x_b,e_q) - M)) - 30*cos(x_b,e_{xe_b}) ]

where e is the circular queue after the (sequential, data-dependent) update.

Split of labor:
  host: the integer queue-update bookkeeping, the per-pid masked means
    (normalized exactly, fp8-quantized — they become ordinary queue columns),
    the target cosines t30_b (exact f64 dot with the normalized mean), and
    the final log/mean. The heavy B x Q x D cosine matmul and the B x Q
    exponentials run on the 8 cores.
  device (per core, tensor-parallel over Q): 2016 queue columns (32 window
    slots + up to 1984 good non-window slots, zero-padded). 32 b-tiles of
    fp8 DoubleRow matmuls fill two PSUM tiles (psmA [128,1024] + psmD
    [128,992] — separate tiles so their readers aren't serialized by the
    tile framework's same-tile reader chaining) with cosines; the
    exponentials are split across three engines:
      * ACT: native Exp (scale=30, bias=-M) on psmA with accum_out giving
        that range's row-sum directly.
      * DVE: Schraudolph exp on psmD — one fused mult+add tensor_scalar
        emitting int16 bf16-bit-patterns (exp(z) ~= bitcast_bf16(
        rint(z*128*log2e + 16256 + C)), C calibrated so the softmax-sum
        error is ~1e-4).
      * Pool (gpsimd): pairwise halving-add of the bf16 exps (SBUF-only
        engine), then DVE row-sums the halved row in 4x mode one tile
        later (so the Pool round-trip never stalls the DVE stream).
Host: S_b = sum_c (sA + sD)_c - n_zero*e^-M;  loss = mean(M + log S_b - t30_b).
"""

import os
import sys

import numpy as np

for _p in ("/opt/trn_rl_repo", "/root/.axon_site/_ro/trn_rl_repo"):
    if os.path.isdir(_p) and _p not in sys.path:
        sys.path.insert(0, _p)

B, D, Q, U = 4096, 512, 16384, 256
N_CORES = 8
UC = U // N_CORES           # 32 window slots per core
F_A = 1024                  # ACT exp cols (psmA, bank-aligned)
F_D = 992                   # DVE schraudolph cols (psmD)
QSC = F_A + F_D             # 2016 queue columns per core
NW = QSC - UC               # non-window columns per core (zero-padded)
HF = F_D // 2
MT = B // 128               # 32 b-tiles
MC = 4                      # b-tiles per xt DMA batch
KD = 2                      # matmul contraction chunks (DoubleRow pairs)
SUB = D // (128 * KD)       # 2
OIM_SCALAR = 30.0
M_BIAS = 30.0               # logits are <= 30 (both sides unit-norm)
IGNORE = -1
NPOOL_TAIL = 1              # last tiles skip Pool (shorter drain chain)

# Schraudolph-exp constants for exp(30*c - 30) emitted as bf16 bit patterns:
# i16 = rint(c*SCH_A + SCH_B); SCH_B holds the -30 bias, the bf16 exponent
# offset (127<<7) and the calibration constant C=-7.368 (zero weighted error
# over the cosine distribution of random unit vectors at D=512).
SCH_A = 5539.948957013619
SCH_B = 10708.683087674835

_PROG_CACHE = {}

# psum/rhs column chunks: (tile, col0, col1) — each is one PSUM bank and one
# matmul accumulation group. psmD first so the DVE exp chain starts early.
_CHUNKS = [("D", 0, 512), ("D", 512, F_D), ("A", 0, 512), ("A", 512, F_A)]


def _build_program():
    import concourse.bacc as bacc
    import concourse.tile as tile
    from concourse import mybir

    f32 = mybir.dt.float32
    i16 = mybir.dt.int16
    bf16 = mybir.dt.bfloat16
    fp8 = mybir.dt.float8e4
    AF = mybir.ActivationFunctionType
    OP = mybir.AluOpType
    DR = mybir.MatmulPerfMode.DoubleRow

    nc = bacc.Bacc("TRN2", target_bir_lowering=False, debug=False,
                   num_devices=N_CORES)

    xt_d = nc.dram_tensor("xt", [B, D], fp8, kind="ExternalInput").ap()
    emb_d = nc.dram_tensor("emb", [128, (D // 128) * QSC], fp8,
                           kind="ExternalInput").ap()
    sacc_d = nc.dram_tensor("sacc", [128, 2 * MT], f32,
                            kind="ExternalOutput").ap()

    with tile.TileContext(nc) as tc:
        with (
            tc.tile_pool(name="singles", bufs=1) as singles,
            tc.tile_pool(name="twork", bufs=3) as twork,
            tc.tile_pool(name="ework", bufs=3) as ework,
        ):
            biasM = singles.tile([128, 1], f32)
            nc.vector.memset(biasM, -M_BIAS)
            # preload the Exp activation table while DMAs stream
            junk1 = singles.tile([128, 1], f32)
            nc.scalar.activation(out=junk1, in_=biasM, func=AF.Exp)

            sacc = singles.tile([128, 2 * MT], f32)

            # queue columns, all from DMA (window means host-computed).
            # embD first: DVE is the critical engine, so psmD's matmuls
            # must start earliest; embA last (ACT has slack to absorb it).
            # One DMA per region keeps runs >= 512B (no descriptor latency
            # penalty).
            embt = singles.tile([128, KD, SUB, QSC], fp8)
            emb4 = emb_d.rearrange("p (a b c) -> p a b c", a=KD, b=SUB)
            nc.sync.dma_start(out=embt[:, :, :, 0:F_D],
                              in_=emb4[:, :, :, 0:F_D])
            first_xt = twork.tile([128, 1, D], fp8, tag="tl1")
            nc.sync.dma_start(
                out=first_xt,
                in_=xt_d[0:128, :].rearrange("(j p) d -> p j d", j=1))
            nc.sync.dma_start(out=embt[:, :, :, F_D:QSC],
                              in_=emb4[:, :, :, F_D:QSC])

            batches = [(0, 1)] + [(1 + k * MC, min(1 + (k + 1) * MC, MT))
                                  for k in range((MT - 1 + MC - 1) // MC)]
            pend = []           # (m, tile, ncols) rows awaiting the DVE sum

            def emit_sum(item):
                m_, t_, n_ = item
                sj = ework.tile([128, n_], bf16, tag="sj")
                nc.vector.tensor_scalar(
                    out=sj, in0=t_, scalar1=1.0, scalar2=None,
                    op0=OP.mult, op1=OP.add,
                    accum_out=sacc[:, MT + m_:MT + m_ + 1])

            with tc.tile_pool(name="psC", bufs=2, space="PSUM") as psC:
                for (b0, b1) in batches:
                    nb = b1 - b0
                    if b0 == 0:
                        tl4 = first_xt
                    else:
                        tl4 = twork.tile([128, nb, D], fp8, tag=f"tl{nb}")
                        nc.sync.dma_start(
                            out=tl4,
                            in_=xt_d[b0 * 128:b1 * 128, :]
                            .rearrange("(j p) d -> p j d", j=nb))
                    for j in range(nb):
                        m = b0 + j
                        tlm = tl4[:, j].rearrange("p (a b c) -> p a b c",
                                                  a=KD, b=SUB)
                        psmA = psC.tile([128, F_A], f32, tag="psmA")
                        psmD = psC.tile([128, F_D], f32, tag="psmD")
                        for (t, p0, p1) in _CHUNKS:
                            ps, o = (psmD, 0) if t == "D" else (psmA, F_D)
                            for kd in range(KD):
                                nc.tensor.matmul(ps[:, p0:p1], tlm[:, kd],
                                                 embt[:, kd, :,
                                                      o + p0:o + p1],
                                                 start=(kd == 0),
                                                 stop=(kd == KD - 1),
                                                 perf_mode=DR)
                        # DVE: schraudolph exp of psmD as bf16 bit patterns
                        eDP = ework.tile([128, F_D], i16, tag="eDP")
                        nc.vector.tensor_scalar(out=eDP, in0=psmD,
                                                scalar1=SCH_A, scalar2=SCH_B,
                                                op0=OP.mult, op1=OP.add)
                        # ACT: exp + row-sum of psmA
                        scrA = ework.tile([128, F_A], bf16, tag="scrA")
                        nc.scalar.activation(out=scrA, in_=psmA,
                                             func=AF.Exp, bias=biasM,
                                             scale=OIM_SCALAR,
                                             accum_out=sacc[:, m:m + 1])
                        ebf = eDP.bitcast(bf16)
                        if m < MT - NPOOL_TAIL:
                            # Pool: halve by pairwise add; DVE sums halved
                            # row one tile later (Pool latency overlaps the
                            # next schraudolph instead of stalling DVE)
                            ph = ework.tile([128, HF], bf16, tag="ph")
                            nc.gpsimd.tensor_tensor(out=ph, in0=ebf[:, 0:HF],
                                                    in1=ebf[:, HF:],
                                                    op=OP.add)
                            pend.append((m, ph, HF))
                        else:
                            # drain tail: skip the Pool round-trip
                            pend.append((m, ebf, F_D))
                        if len(pend) > 1:
                            emit_sum(pend.pop(0))
                while pend:
                    emit_sum(pend.pop(0))

            nc.sync.dma_start(out=sacc_d, in_=sacc)

    nc.compile()
    return nc


def _host_bookkeeping(labels, label_cq, header_cq):
    """Mirror the reference's integer-only queue-update semantics."""
    labels = np.asarray(labels).astype(np.int64)
    lab = np.asarray(label_cq).astype(np.int64).copy()
    h0 = int(np.asarray(header_cq))

    uq = np.unique(labels)
    if uq.size < U:
        uniq = np.concatenate([uq, np.full(U - uq.size, uq.min(), np.int64)])
    else:
        uniq = uq[:U]

    emb_src = np.full(Q, -1, np.int64)   # >=0: window slot written by uniq u
    h = h0 % Q
    for u in range(U):
        y = uniq[u]
        m = lab == y
        i = int(np.argmax(m)) if m.any() else 0
        inval = bool(m.any()) and (i != h)
        emb_src[h] = u
        lab[h] = y
        if inval:
            lab[i] = IGNORE
        h = (h + 1) % Q

    good = lab != IGNORE
    goodidx = np.flatnonzero(good)
    gl = lab[goodidx]
    vals, first = np.unique(gl, return_index=True)
    pos = np.searchsorted(vals, labels)
    assert np.all(vals[np.clip(pos, 0, vals.size - 1)] == labels), \
        "batch label missing from queue"
    xe = goodidx[first[pos]]
    return uniq, emb_src, good, xe


def _prepare(inputs, labels, emb_cq, label_cq, header_cq):
    import ml_dtypes
    e_dt = ml_dtypes.float8_e4m3

    x = np.ascontiguousarray(np.asarray(inputs, np.float32))
    emb_cq = np.ascontiguousarray(np.asarray(emb_cq, np.float32))
    labels_i = np.asarray(labels).astype(np.int64)

    uniq, emb_src, good, xe = _host_bookkeeping(labels, label_cq, header_cq)

    # per-pid means over the batch (sorted-group reduceat), normalized exactly
    order = np.argsort(labels_i, kind="stable")
    ls = labels_i[order]
    starts = np.flatnonzero(np.r_[True, ls[1:] != ls[:-1]])
    vals = ls[starts]
    sums = np.add.reduceat(x[order].astype(np.float64), starts, axis=0)
    counts = np.diff(np.r_[starts, ls.size])[:, None]
    means = sums / counts
    means /= np.maximum(np.linalg.norm(means, axis=1, keepdims=True), 1e-12)

    # window columns in slot order; invalidated window slots become zeros
    h0 = int(np.asarray(header_cq)) % Q
    wslot = (h0 + np.arange(U)) % Q
    u_valid = good[wslot]
    widx_of_uniq = np.searchsorted(vals, uniq)
    win_emb = means[widx_of_uniq] * u_valid[:, None]          # [U, D] f64

    # exact target cosines on the host
    xn64 = x.astype(np.float64)
    xn64 /= np.maximum(np.linalg.norm(xn64, axis=1, keepdims=True), 1e-12)
    w_idx = emb_src[xe]                      # target window index, -1=extra
    tgt = np.empty((B, D), np.float64)
    winrows = w_idx >= 0
    tgt[winrows] = means[widx_of_uniq[np.clip(w_idx, 0, U - 1)][winrows]]
    if (~winrows).any():
        eb = emb_cq[xe[~winrows]].astype(np.float64)
        tgt[~winrows] = eb
    t30 = OIM_SCALAR * np.einsum("bd,bd->b", xn64, tgt)

    # d-major row-normalized fp8 x for the logits lhsT (per-row 1/|x| folded
    # into the quantization)
    xn = (xn64.astype(np.float32)).astype(e_dt)
    Y = xn.reshape(MT, 128, KD, SUB, 128)
    xt = np.ascontiguousarray(Y.transpose(0, 4, 2, 3, 1).reshape(B, D))

    # queue columns per core: 32 window means + good non-window slots.
    # device layout: cols [0:F_D] = psmD range, [F_D:QSC] = psmA range;
    # the window means go at the start of the psmA range (arbitrary).
    nonwin = np.flatnonzero(good & (emb_src < 0))
    assert nonwin.size <= N_CORES * NW, "queue overflow vs compiled shape"
    parts = np.array_split(nonwin, N_CORES)
    in_maps = []
    n_zero = int((~u_valid).sum())
    for c in range(N_CORES):
        cols = parts[c]
        n_zero += NW - cols.size
        E = np.zeros((QSC, D), np.float32)
        E[:F_D] = emb_cq[cols[:F_D]]
        E[F_D:F_D + UC] = win_emb[c * UC:(c + 1) * UC]
        rest = cols[F_D:]
        E[F_D + UC:F_D + UC + rest.size] = emb_cq[rest]
        Z = E.astype(e_dt).reshape(QSC, KD, SUB, 128)
        embp = np.ascontiguousarray(
            Z.transpose(3, 1, 2, 0).reshape(128, KD * SUB * QSC))
        in_maps.append({"xt": xt, "emb": embp})
    return in_maps, t30, n_zero


def _combine(res_list, t30, n_zero):
    S = np.zeros(B, np.float64)
    for r in res_list:
        sa = r["sacc"].astype(np.float64)
        S += sa[:, :MT].T.reshape(B) + sa[:, MT:].T.reshape(B)
    S -= n_zero * np.exp(-float(M_BIAS))
    loss = np.mean(M_BIAS + np.log(S) - t30)
    return np.array(loss, dtype=np.float32)


def kernel(inputs, labels, emb_cq, label_cq, age_cq, header_cq):
    from concourse.bass_utils import run_bass_kernel_spmd

    in_maps, t30, n_zero = _prepare(inputs, labels, emb_cq, label_cq,
                                    header_cq)

    if "prog" not in _PROG_CACHE:
        _PROG_CACHE["prog"] = _build_program()
    nc = _PROG_CACHE["prog"]

    res = run_bass_kernel_spmd(nc, in_maps, core_ids=list(range(N_CORES)))
    return _combine(res.results, t30, n_zero)


# revision 10
# speedup vs baseline: 1.3691x; 1.0132x over previous
"""OIM loss with circular queue — Trainium2 Bass kernel (8 NeuronCores).

loss = mean_b [ M + log(sum_{q good} exp(30*cos(x_b,e_q) - M)) - 30*cos(x_b,e_{xe_b}) ]

where e is the circular queue after the (sequential, data-dependent) update.

Split of labor:
  host: the integer queue-update bookkeeping, the per-pid masked means
    (normalized exactly, fp8-quantized — they become ordinary queue columns),
    the target cosines t30_b (exact f64 dot with the normalized mean), and
    the final log/mean. The heavy B x Q x D cosine matmul and the B x Q
    exponentials run on the 8 cores.
  device (per core, tensor-parallel over Q): 2016 queue columns (32 window
    slots + up to 1984 good non-window slots, zero-padded). 32 b-tiles of
    fp8 DoubleRow matmuls fill two PSUM tiles (psmA [128,1024] + psmD
    [128,992] — separate tiles so their readers aren't serialized by the
    tile framework's same-tile reader chaining) with cosines; the
    exponentials are split across three engines:
      * ACT: native Exp (scale=30, bias=-M) on psmA with accum_out giving
        that range's row-sum directly.
      * DVE: Schraudolph exp on psmD — one fused mult+add tensor_scalar
        emitting int16 bf16-bit-patterns (exp(z) ~= bitcast_bf16(
        rint(z*128*log2e + 16256 + C)), C calibrated so the softmax-sum
        error is ~1e-4).
      * Pool (gpsimd): pairwise halving-add of the bf16 exps (SBUF-only
        engine), then DVE row-sums the halved row in 4x mode one tile
        later (so the Pool round-trip never stalls the DVE stream).
Host: S_b = sum_c (sA + sD)_c - n_zero*e^-M;  loss = mean(M + log S_b - t30_b).
"""

import os
import sys

import numpy as np

for _p in ("/opt/trn_rl_repo", "/root/.axon_site/_ro/trn_rl_repo"):
    if os.path.isdir(_p) and _p not in sys.path:
        sys.path.insert(0, _p)

B, D, Q, U = 4096, 512, 16384, 256
N_CORES = 8
UC = U // N_CORES           # 32 window slots per core
F_A = 1024                  # ACT exp cols (psmA, bank-aligned)
F_D = 992                   # DVE schraudolph cols (psmD)
QSC = F_A + F_D             # 2016 queue columns per core
NW = QSC - UC               # non-window columns per core (zero-padded)
HF = F_D // 2
MT = B // 128               # 32 b-tiles
MC = 4                      # b-tiles per xt DMA batch
KD = 2                      # matmul contraction chunks (DoubleRow pairs)
SUB = D // (128 * KD)       # 2
OIM_SCALAR = 30.0
M_BIAS = 30.0               # logits are <= 30 (both sides unit-norm)
IGNORE = -1
NPOOL_TAIL = 1              # last tiles skip Pool (shorter drain chain)

# Schraudolph-exp constants for exp(30*c - 30) emitted as bf16 bit patterns:
# i16 = rint(c*SCH_A + SCH_B); SCH_B holds the -30 bias, the bf16 exponent
# offset (127<<7) and the calibration constant C=-7.368 (zero weighted error
# over the cosine distribution of random unit vectors at D=512).
SCH_A = 5539.948957013619
SCH_B = 10708.683087674835

_PROG_CACHE = {}

# psum/rhs column chunks: (tile, col0, col1) — each is one PSUM bank and one
# matmul accumulation group. psmD first so the DVE exp chain starts early.
_CHUNKS = [("D", 0, 512), ("D", 512, F_D), ("A", 0, 512), ("A", 512, F_A)]


def _build_program():
    import concourse.bacc as bacc
    import concourse.tile as tile
    from concourse import mybir

    f32 = mybir.dt.float32
    i16 = mybir.dt.int16
    bf16 = mybir.dt.bfloat16
    fp8 = mybir.dt.float8e4
    AF = mybir.ActivationFunctionType
    OP = mybir.AluOpType
    DR = mybir.MatmulPerfMode.DoubleRow

    nc = bacc.Bacc("TRN2", target_bir_lowering=False, debug=False,
                   num_devices=N_CORES)

    xt_d = nc.dram_tensor("xt", [B, D], fp8, kind="ExternalInput").ap()
    emb_d = nc.dram_tensor("emb", [128, (D // 128) * QSC], fp8,
                           kind="ExternalInput").ap()
    sacc_d = nc.dram_tensor("sacc", [128, 2 * MT], f32,
                            kind="ExternalOutput").ap()

    with tile.TileContext(nc) as tc:
        with (
            tc.tile_pool(name="singles", bufs=1) as singles,
            tc.tile_pool(name="twork", bufs=3) as twork,
            tc.tile_pool(name="ework", bufs=3) as ework,
        ):
            biasM = singles.tile([128, 1], f32)
            nc.vector.memset(biasM, -M_BIAS)
            # preload the Exp activation table while DMAs stream
            junk1 = singles.tile([128, 1], f32)
            nc.scalar.activation(out=junk1, in_=biasM, func=AF.Exp)

            sacc = singles.tile([128, 2 * MT], f32)

            # queue columns, all from DMA (window means host-computed).
            # embD first: DVE is the critical engine, so psmD's matmuls
            # must start earliest; embA last (ACT has slack to absorb it).
            # One DMA per region keeps runs >= 512B (no descriptor latency
            # penalty).
            embt = singles.tile([128, KD, SUB, QSC], fp8)
            emb4 = emb_d.rearrange("p (a b c) -> p a b c", a=KD, b=SUB)
            nc.sync.dma_start(out=embt[:, :, :, 0:F_D],
                              in_=emb4[:, :, :, 0:F_D])
            first_xt = twork.tile([128, 1, D], fp8, tag="tl1")
            nc.sync.dma_start(
                out=first_xt,
                in_=xt_d[0:128, :].rearrange("(j p) d -> p j d", j=1))
            nc.sync.dma_start(out=embt[:, :, :, F_D:QSC],
                              in_=emb4[:, :, :, F_D:QSC])

            # PE p-state warmup: the ramp (0.65 -> 2.4 GHz over 3us of
            # continuous busy) would otherwise burn the first ~5 tiles at
            # half speed. Dummy matmuls into the first psmA rotation buffer
            # keep the PE busy until the input DMAs land; the first real
            # matmul queues behind them with no idle gap, so the ramp
            # carries over.
            warm = singles.tile([128, 256], fp8)
            nc.gpsimd.memset(warm, 0)

            batches = [(0, 1)] + [(1 + k * MC, min(1 + (k + 1) * MC, MT))
                                  for k in range((MT - 1 + MC - 1) // MC)]
            pend = []           # (m, tile, ncols) rows awaiting the DVE sum

            def emit_sum(item):
                m_, t_, n_ = item
                sj = ework.tile([128, n_], bf16, tag="sj")
                nc.vector.tensor_scalar(
                    out=sj, in0=t_, scalar1=1.0, scalar2=None,
                    op0=OP.mult, op1=OP.add,
                    accum_out=sacc[:, MT + m_:MT + m_ + 1])

            with tc.tile_pool(name="psC", bufs=2, space="PSUM") as psC:
                psW = psC.tile([128, F_A], f32, tag="psmA")
                for w in range(15):
                    nc.tensor.matmul(psW[:, 0:256], warm[:, 0:128],
                                     warm, start=True, stop=True)
                for (b0, b1) in batches:
                    nb = b1 - b0
                    if b0 == 0:
                        tl4 = first_xt
                    else:
                        tl4 = twork.tile([128, nb, D], fp8, tag=f"tl{nb}")
                        nc.sync.dma_start(
                            out=tl4,
                            in_=xt_d[b0 * 128:b1 * 128, :]
                            .rearrange("(j p) d -> p j d", j=nb))
                    for j in range(nb):
                        m = b0 + j
                        tlm = tl4[:, j].rearrange("p (a b c) -> p a b c",
                                                  a=KD, b=SUB)
                        psmA = psC.tile([128, F_A], f32, tag="psmA")
                        psmD = psC.tile([128, F_D], f32, tag="psmD")
                        for (t, p0, p1) in _CHUNKS:
                            ps, o = (psmD, 0) if t == "D" else (psmA, F_D)
                            for kd in range(KD):
                                nc.tensor.matmul(ps[:, p0:p1], tlm[:, kd],
                                                 embt[:, kd, :,
                                                      o + p0:o + p1],
                                                 start=(kd == 0),
                                                 stop=(kd == KD - 1),
                                                 perf_mode=DR)
                        # DVE: schraudolph exp of psmD as bf16 bit patterns
                        eDP = ework.tile([128, F_D], i16, tag="eDP")
                        nc.vector.tensor_scalar(out=eDP, in0=psmD,
                                                scalar1=SCH_A, scalar2=SCH_B,
                                                op0=OP.mult, op1=OP.add)
                        # ACT: exp + row-sum of psmA
                        scrA = ework.tile([128, F_A], bf16, tag="scrA")
                        nc.scalar.activation(out=scrA, in_=psmA,
                                             func=AF.Exp, bias=biasM,
                                             scale=OIM_SCALAR,
                                             accum_out=sacc[:, m:m + 1])
                        ebf = eDP.bitcast(bf16)
                        if m < MT - NPOOL_TAIL:
                            # Pool: halve by pairwise add; DVE sums halved
                            # row one tile later (Pool latency overlaps the
                            # next schraudolph instead of stalling DVE)
                            ph = ework.tile([128, HF], bf16, tag="ph")
                            nc.gpsimd.tensor_tensor(out=ph, in0=ebf[:, 0:HF],
                                                    in1=ebf[:, HF:],
                                                    op=OP.add)
                            pend.append((m, ph, HF))
                        else:
                            # drain tail: skip the Pool round-trip
                            pend.append((m, ebf, F_D))
                        if len(pend) > 1:
                            emit_sum(pend.pop(0))
                while pend:
                    emit_sum(pend.pop(0))

            nc.sync.dma_start(out=sacc_d, in_=sacc)

    nc.compile()
    return nc


def _host_bookkeeping(labels, label_cq, header_cq):
    """Mirror the reference's integer-only queue-update semantics."""
    labels = np.asarray(labels).astype(np.int64)
    lab = np.asarray(label_cq).astype(np.int64).copy()
    h0 = int(np.asarray(header_cq))

    uq = np.unique(labels)
    if uq.size < U:
        uniq = np.concatenate([uq, np.full(U - uq.size, uq.min(), np.int64)])
    else:
        uniq = uq[:U]

    emb_src = np.full(Q, -1, np.int64)   # >=0: window slot written by uniq u
    h = h0 % Q
    for u in range(U):
        y = uniq[u]
        m = lab == y
        i = int(np.argmax(m)) if m.any() else 0
        inval = bool(m.any()) and (i != h)
        emb_src[h] = u
        lab[h] = y
        if inval:
            lab[i] = IGNORE
        h = (h + 1) % Q

    good = lab != IGNORE
    goodidx = np.flatnonzero(good)
    gl = lab[goodidx]
    vals, first = np.unique(gl, return_index=True)
    pos = np.searchsorted(vals, labels)
    assert np.all(vals[np.clip(pos, 0, vals.size - 1)] == labels), \
        "batch label missing from queue"
    xe = goodidx[first[pos]]
    return uniq, emb_src, good, xe


def _prepare(inputs, labels, emb_cq, label_cq, header_cq):
    import ml_dtypes
    e_dt = ml_dtypes.float8_e4m3

    x = np.ascontiguousarray(np.asarray(inputs, np.float32))
    emb_cq = np.ascontiguousarray(np.asarray(emb_cq, np.float32))
    labels_i = np.asarray(labels).astype(np.int64)

    uniq, emb_src, good, xe = _host_bookkeeping(labels, label_cq, header_cq)

    # per-pid means over the batch (sorted-group reduceat), normalized exactly
    order = np.argsort(labels_i, kind="stable")
    ls = labels_i[order]
    starts = np.flatnonzero(np.r_[True, ls[1:] != ls[:-1]])
    vals = ls[starts]
    sums = np.add.reduceat(x[order].astype(np.float64), starts, axis=0)
    counts = np.diff(np.r_[starts, ls.size])[:, None]
    means = sums / counts
    means /= np.maximum(np.linalg.norm(means, axis=1, keepdims=True), 1e-12)

    # window columns in slot order; invalidated window slots become zeros
    h0 = int(np.asarray(header_cq)) % Q
    wslot = (h0 + np.arange(U)) % Q
    u_valid = good[wslot]
    widx_of_uniq = np.searchsorted(vals, uniq)
    win_emb = means[widx_of_uniq] * u_valid[:, None]          # [U, D] f64

    # exact target cosines on the host
    xn64 = x.astype(np.float64)
    xn64 /= np.maximum(np.linalg.norm(xn64, axis=1, keepdims=True), 1e-12)
    w_idx = emb_src[xe]                      # target window index, -1=extra
    tgt = np.empty((B, D), np.float64)
    winrows = w_idx >= 0
    tgt[winrows] = means[widx_of_uniq[np.clip(w_idx, 0, U - 1)][winrows]]
    if (~winrows).any():
        eb = emb_cq[xe[~winrows]].astype(np.float64)
        tgt[~winrows] = eb
    t30 = OIM_SCALAR * np.einsum("bd,bd->b", xn64, tgt)

    # d-major row-normalized fp8 x for the logits lhsT (per-row 1/|x| folded
    # into the quantization)
    xn = (xn64.astype(np.float32)).astype(e_dt)
    Y = xn.reshape(MT, 128, KD, SUB, 128)
    xt = np.ascontiguousarray(Y.transpose(0, 4, 2, 3, 1).reshape(B, D))

    # queue columns per core: 32 window means + good non-window slots.
    # device layout: cols [0:F_D] = psmD range, [F_D:QSC] = psmA range;
    # the window means go at the start of the psmA range (arbitrary).
    nonwin = np.flatnonzero(good & (emb_src < 0))
    assert nonwin.size <= N_CORES * NW, "queue overflow vs compiled shape"
    parts = np.array_split(nonwin, N_CORES)
    in_maps = []
    n_zero = int((~u_valid).sum())
    for c in range(N_CORES):
        cols = parts[c]
        n_zero += NW - cols.size
        E = np.zeros((QSC, D), np.float32)
        E[:F_D] = emb_cq[cols[:F_D]]
        E[F_D:F_D + UC] = win_emb[c * UC:(c + 1) * UC]
        rest = cols[F_D:]
        E[F_D + UC:F_D + UC + rest.size] = emb_cq[rest]
        Z = E.astype(e_dt).reshape(QSC, KD, SUB, 128)
        embp = np.ascontiguousarray(
            Z.transpose(3, 1, 2, 0).reshape(128, KD * SUB * QSC))
        in_maps.append({"xt": xt, "emb": embp})
    return in_maps, t30, n_zero


def _combine(res_list, t30, n_zero):
    S = np.zeros(B, np.float64)
    for r in res_list:
        sa = r["sacc"].astype(np.float64)
        S += sa[:, :MT].T.reshape(B) + sa[:, MT:].T.reshape(B)
    S -= n_zero * np.exp(-float(M_BIAS))
    loss = np.mean(M_BIAS + np.log(S) - t30)
    return np.array(loss, dtype=np.float32)


def kernel(inputs, labels, emb_cq, label_cq, age_cq, header_cq):
    from concourse.bass_utils import run_bass_kernel_spmd

    in_maps, t30, n_zero = _prepare(inputs, labels, emb_cq, label_cq,
                                    header_cq)

    if "prog" not in _PROG_CACHE:
        _PROG_CACHE["prog"] = _build_program()
    nc = _PROG_CACHE["prog"]

    res = run_bass_kernel_spmd(nc, in_maps, core_ids=list(range(N_CORES)))
    return _combine(res.results, t30, n_zero)


# revision 12
# speedup vs baseline: 1.4033x; 1.0250x over previous
"""OIM loss with circular queue — Trainium2 Bass kernel (8 NeuronCores).

loss = mean_b [ M + log(sum_{q good} exp(30*cos(x_b,e_q) - M)) - 30*cos(x_b,e_{xe_b}) ]

where e is the circular queue after the (sequential, data-dependent) update.

Split of labor:
  host: the integer queue-update bookkeeping, the per-pid masked means
    (normalized exactly, fp8-quantized — they become ordinary queue columns),
    the target cosines t30_b (exact f64 dot with the normalized mean), and
    the final log/mean. The heavy B x Q x D cosine matmul and the B x Q
    exponentials run on the 8 cores.
  device (per core, tensor-parallel over Q): 2016 queue columns (32 window
    slots + up to 1984 good non-window slots, zero-padded). 32 b-tiles of
    fp8 DoubleRow matmuls fill two PSUM tiles (psmA [128,1024] + psmD
    [128,992] — separate tiles so their readers aren't serialized by the
    tile framework's same-tile reader chaining) with cosines; the
    exponentials are split across three engines:
      * ACT: native Exp (scale=30, bias=-M) on psmA with accum_out giving
        that range's row-sum directly.
      * DVE: Schraudolph exp on psmD — one fused mult+add tensor_scalar
        emitting int16 bf16-bit-patterns (exp(z) ~= bitcast_bf16(
        rint(z*128*log2e + 16256 + C)), C calibrated so the softmax-sum
        error is ~1e-4).
      * Pool (gpsimd): pairwise halving-add of the bf16 exps (SBUF-only
        engine), then DVE row-sums the halved row in 4x mode one tile
        later (so the Pool round-trip never stalls the DVE stream).
Host: S_b = sum_c (sA + sD)_c - n_zero*e^-M;  loss = mean(M + log S_b - t30_b).
"""

import os
import sys

import numpy as np

for _p in ("/opt/trn_rl_repo", "/root/.axon_site/_ro/trn_rl_repo"):
    if os.path.isdir(_p) and _p not in sys.path:
        sys.path.insert(0, _p)

B, D, Q, U = 4096, 512, 16384, 256
N_CORES = 8
UC = U // N_CORES           # 32 window slots per core
F_A = 1024                  # ACT exp cols (psmA, bank-aligned)
F_D = 992                   # DVE schraudolph cols (psmD)
QSC = F_A + F_D             # 2016 queue columns per core
NW = QSC - UC               # non-window columns per core (zero-padded)
HF = F_D // 2
MT = B // 128               # 32 b-tiles
MC = 4                      # b-tiles per xt DMA batch
KD = 2                      # matmul contraction chunks (DoubleRow pairs)
SUB = D // (128 * KD)       # 2
OIM_SCALAR = 30.0
M_BIAS = 30.0               # logits are <= 30 (both sides unit-norm)
IGNORE = -1
NPOOL_TAIL = 1              # last tiles skip Pool (shorter drain chain)

# Schraudolph-exp constants for exp(30*c - 30) emitted as bf16 bit patterns:
# i16 = rint(c*SCH_A + SCH_B); SCH_B holds the -30 bias, the bf16 exponent
# offset (127<<7) and the calibration constant C=-7.368 (zero weighted error
# over the cosine distribution of random unit vectors at D=512).
SCH_A = 5539.948957013619
SCH_B = 10708.683087674835

_PROG_CACHE = {}

# psum/rhs column chunks: (tile, col0, col1) — each is one PSUM bank and one
# matmul accumulation group. psmD first so the DVE exp chain starts early.
_CHUNKS = [("D", 0, 512), ("D", 512, F_D), ("A", 0, 512), ("A", 512, F_A)]


def _build_program():
    import concourse.bacc as bacc
    import concourse.tile as tile
    from concourse import mybir

    f32 = mybir.dt.float32
    i16 = mybir.dt.int16
    bf16 = mybir.dt.bfloat16
    fp8 = mybir.dt.float8e4
    AF = mybir.ActivationFunctionType
    OP = mybir.AluOpType
    DR = mybir.MatmulPerfMode.DoubleRow

    nc = bacc.Bacc("TRN2", target_bir_lowering=False, debug=False,
                   num_devices=N_CORES)

    xt_d = nc.dram_tensor("xt", [B, D], fp8, kind="ExternalInput").ap()
    emb_d = nc.dram_tensor("emb", [128, (D // 128) * QSC], fp8,
                           kind="ExternalInput").ap()
    sacc_d = nc.dram_tensor("sacc", [128, 2 * MT], f32,
                            kind="ExternalOutput").ap()

    with tile.TileContext(nc) as tc:
        with (
            tc.tile_pool(name="singles", bufs=1) as singles,
            tc.tile_pool(name="twork", bufs=3) as twork,
            tc.tile_pool(name="ework", bufs=3) as ework,
        ):
            biasM = singles.tile([128, 1], f32)
            nc.vector.memset(biasM, -M_BIAS)
            # preload the Exp activation table while DMAs stream
            junk1 = singles.tile([128, 1], f32)
            nc.scalar.activation(out=junk1, in_=biasM, func=AF.Exp)

            sacc = singles.tile([128, 2 * MT], f32)

            # queue columns, all from DMA (window means host-computed).
            # embD first: DVE is the critical engine, so psmD's matmuls
            # must start earliest; embA last (ACT has slack to absorb it).
            # One DMA per region keeps runs >= 512B (no descriptor latency
            # penalty).
            embt = singles.tile([128, KD, SUB, QSC], fp8)
            emb4 = emb_d.rearrange("p (a b c) -> p a b c", a=KD, b=SUB)
            nc.sync.dma_start(out=embt[:, :, :, 0:F_D],
                              in_=emb4[:, :, :, 0:F_D])
            first_xt = twork.tile([128, 1, D], fp8, tag="tl1")
            nc.sync.dma_start(
                out=first_xt,
                in_=xt_d[0:128, :].rearrange("(j p) d -> p j d", j=1))
            # second xt batch (tiles 1-4) before embA: the D-side pipeline
            # (the critical DVE chain) must never wait behind embA's 1.5us
            # transfer; the delayed A-side absorbs embA's late arrival.
            second_xt = twork.tile([128, MC, D], fp8, tag=f"tl{MC}")
            nc.sync.dma_start(
                out=second_xt,
                in_=xt_d[128:128 * (1 + MC), :]
                .rearrange("(j p) d -> p j d", j=MC))
            nc.sync.dma_start(out=embt[:, :, :, F_D:QSC],
                              in_=emb4[:, :, :, F_D:QSC])

            # PE p-state warmup: the ramp (0.65 -> 2.4 GHz over 3us of
            # continuous busy) would otherwise burn the first ~5 tiles at
            # half speed. Dummy matmuls into the first psmA rotation buffer
            # keep the PE busy until the input DMAs land; the first real
            # matmul queues behind them with no idle gap, so the ramp
            # carries over.
            warm = singles.tile([128, 256], fp8)
            nc.gpsimd.memset(warm, 0)

            batches = [(0, 1)] + [(1 + k * MC, min(1 + (k + 1) * MC, MT))
                                  for k in range((MT - 1 + MC - 1) // MC)]
            pend = []           # (m, tile, ncols) rows awaiting the DVE sum

            def emit_sum(item):
                m_, t_, n_ = item
                sj = ework.tile([128, n_], bf16, tag="sj")
                nc.vector.tensor_scalar(
                    out=sj, in0=t_, scalar1=1.0, scalar2=None,
                    op0=OP.mult, op1=OP.add,
                    accum_out=sacc[:, MT + m_:MT + m_ + 1])

            with tc.tile_pool(name="psC", bufs=2, space="PSUM") as psC:
                pend_A = []     # (m, tlm) awaiting the delayed A side

                def emit_A(item):
                    m_, tlm_ = item
                    psmA = psC.tile([128, F_A], f32, tag="psmA")
                    for (p0, p1) in [(0, 512), (512, F_A)]:
                        for kd in range(KD):
                            nc.tensor.matmul(psmA[:, p0:p1], tlm_[:, kd],
                                             embt[:, kd, :,
                                                  F_D + p0:F_D + p1],
                                             start=(kd == 0),
                                             stop=(kd == KD - 1),
                                             perf_mode=DR)
                    scrA = ework.tile([128, F_A], bf16, tag="scrA")
                    nc.scalar.activation(out=scrA, in_=psmA,
                                         func=AF.Exp, bias=biasM,
                                         scale=OIM_SCALAR,
                                         accum_out=sacc[:, m_:m_ + 1])

                psW = psC.tile([128, F_A], f32, tag="psmA")
                for w in range(15):
                    nc.tensor.matmul(psW[:, 0:256], warm[:, 0:128],
                                     warm, start=True, stop=True)
                for (b0, b1) in batches:
                    nb = b1 - b0
                    if b0 == 0:
                        tl4 = first_xt
                    elif b0 == 1:
                        tl4 = second_xt
                    else:
                        tl4 = twork.tile([128, nb, D], fp8, tag=f"tl{nb}")
                        nc.sync.dma_start(
                            out=tl4,
                            in_=xt_d[b0 * 128:b1 * 128, :]
                            .rearrange("(j p) d -> p j d", j=nb))
                    for j in range(nb):
                        m = b0 + j
                        tlm = tl4[:, j].rearrange("p (a b c) -> p a b c",
                                                  a=KD, b=SUB)
                        psmD = psC.tile([128, F_D], f32, tag="psmD")
                        for (p0, p1) in [(0, 512), (512, F_D)]:
                            for kd in range(KD):
                                nc.tensor.matmul(psmD[:, p0:p1], tlm[:, kd],
                                                 embt[:, kd, :, p0:p1],
                                                 start=(kd == 0),
                                                 stop=(kd == KD - 1),
                                                 perf_mode=DR)
                        # DVE: schraudolph exp of psmD as bf16 bit patterns
                        eDP = ework.tile([128, F_D], i16, tag="eDP")
                        nc.vector.tensor_scalar(out=eDP, in0=psmD,
                                                scalar1=SCH_A, scalar2=SCH_B,
                                                op0=OP.mult, op1=OP.add)
                        # A side (psmA matmuls + ACT exp) delayed one tile:
                        # the D-side pipeline never queues behind embA or
                        # the A matmuls, and ACT's slack absorbs the shift
                        pend_A.append((m, tlm))
                        if len(pend_A) > 1:
                            emit_A(pend_A.pop(0))
                        ebf = eDP.bitcast(bf16)
                        if m < MT - NPOOL_TAIL:
                            # Pool: halve by pairwise add; DVE sums halved
                            # row one tile later (Pool latency overlaps the
                            # next schraudolph instead of stalling DVE)
                            ph = ework.tile([128, HF], bf16, tag="ph")
                            nc.gpsimd.tensor_tensor(out=ph, in0=ebf[:, 0:HF],
                                                    in1=ebf[:, HF:],
                                                    op=OP.add)
                            pend.append((m, ph, HF))
                        else:
                            # drain tail: skip the Pool round-trip
                            pend.append((m, ebf, F_D))
                        if len(pend) > 1:
                            emit_sum(pend.pop(0))
                while pend_A:
                    emit_A(pend_A.pop(0))
                while pend:
                    emit_sum(pend.pop(0))

            nc.sync.dma_start(out=sacc_d, in_=sacc)

    nc.compile()
    return nc


def _host_bookkeeping(labels, label_cq, header_cq):
    """Mirror the reference's integer-only queue-update semantics."""
    labels = np.asarray(labels).astype(np.int64)
    lab = np.asarray(label_cq).astype(np.int64).copy()
    h0 = int(np.asarray(header_cq))

    uq = np.unique(labels)
    if uq.size < U:
        uniq = np.concatenate([uq, np.full(U - uq.size, uq.min(), np.int64)])
    else:
        uniq = uq[:U]

    emb_src = np.full(Q, -1, np.int64)   # >=0: window slot written by uniq u
    h = h0 % Q
    for u in range(U):
        y = uniq[u]
        m = lab == y
        i = int(np.argmax(m)) if m.any() else 0
        inval = bool(m.any()) and (i != h)
        emb_src[h] = u
        lab[h] = y
        if inval:
            lab[i] = IGNORE
        h = (h + 1) % Q

    good = lab != IGNORE
    goodidx = np.flatnonzero(good)
    gl = lab[goodidx]
    vals, first = np.unique(gl, return_index=True)
    pos = np.searchsorted(vals, labels)
    assert np.all(vals[np.clip(pos, 0, vals.size - 1)] == labels), \
        "batch label missing from queue"
    xe = goodidx[first[pos]]
    return uniq, emb_src, good, xe


def _prepare(inputs, labels, emb_cq, label_cq, header_cq):
    import ml_dtypes
    e_dt = ml_dtypes.float8_e4m3

    x = np.ascontiguousarray(np.asarray(inputs, np.float32))
    emb_cq = np.ascontiguousarray(np.asarray(emb_cq, np.float32))
    labels_i = np.asarray(labels).astype(np.int64)

    uniq, emb_src, good, xe = _host_bookkeeping(labels, label_cq, header_cq)

    # per-pid means over the batch (sorted-group reduceat), normalized exactly
    order = np.argsort(labels_i, kind="stable")
    ls = labels_i[order]
    starts = np.flatnonzero(np.r_[True, ls[1:] != ls[:-1]])
    vals = ls[starts]
    sums = np.add.reduceat(x[order].astype(np.float64), starts, axis=0)
    counts = np.diff(np.r_[starts, ls.size])[:, None]
    means = sums / counts
    means /= np.maximum(np.linalg.norm(means, axis=1, keepdims=True), 1e-12)

    # window columns in slot order; invalidated window slots become zeros
    h0 = int(np.asarray(header_cq)) % Q
    wslot = (h0 + np.arange(U)) % Q
    u_valid = good[wslot]
    widx_of_uniq = np.searchsorted(vals, uniq)
    win_emb = means[widx_of_uniq] * u_valid[:, None]          # [U, D] f64

    # exact target cosines on the host
    xn64 = x.astype(np.float64)
    xn64 /= np.maximum(np.linalg.norm(xn64, axis=1, keepdims=True), 1e-12)
    w_idx = emb_src[xe]                      # target window index, -1=extra
    tgt = np.empty((B, D), np.float64)
    winrows = w_idx >= 0
    tgt[winrows] = means[widx_of_uniq[np.clip(w_idx, 0, U - 1)][winrows]]
    if (~winrows).any():
        eb = emb_cq[xe[~winrows]].astype(np.float64)
        tgt[~winrows] = eb
    t30 = OIM_SCALAR * np.einsum("bd,bd->b", xn64, tgt)

    # d-major row-normalized fp8 x for the logits lhsT (per-row 1/|x| folded
    # into the quantization)
    xn = (xn64.astype(np.float32)).astype(e_dt)
    Y = xn.reshape(MT, 128, KD, SUB, 128)
    xt = np.ascontiguousarray(Y.transpose(0, 4, 2, 3, 1).reshape(B, D))

    # queue columns per core: 32 window means + good non-window slots.
    # device layout: cols [0:F_D] = psmD range, [F_D:QSC] = psmA range;
    # the window means go at the start of the psmA range (arbitrary).
    nonwin = np.flatnonzero(good & (emb_src < 0))
    assert nonwin.size <= N_CORES * NW, "queue overflow vs compiled shape"
    parts = np.array_split(nonwin, N_CORES)
    in_maps = []
    n_zero = int((~u_valid).sum())
    for c in range(N_CORES):
        cols = parts[c]
        n_zero += NW - cols.size
        E = np.zeros((QSC, D), np.float32)
        E[:F_D] = emb_cq[cols[:F_D]]
        E[F_D:F_D + UC] = win_emb[c * UC:(c + 1) * UC]
        rest = cols[F_D:]
        E[F_D + UC:F_D + UC + rest.size] = emb_cq[rest]
        Z = E.astype(e_dt).reshape(QSC, KD, SUB, 128)
        embp = np.ascontiguousarray(
            Z.transpose(3, 1, 2, 0).reshape(128, KD * SUB * QSC))
        in_maps.append({"xt": xt, "emb": embp})
    return in_maps, t30, n_zero


def _combine(res_list, t30, n_zero):
    S = np.zeros(B, np.float64)
    for r in res_list:
        sa = r["sacc"].astype(np.float64)
        S += sa[:, :MT].T.reshape(B) + sa[:, MT:].T.reshape(B)
    S -= n_zero * np.exp(-float(M_BIAS))
    loss = np.mean(M_BIAS + np.log(S) - t30)
    return np.array(loss, dtype=np.float32)


def kernel(inputs, labels, emb_cq, label_cq, age_cq, header_cq):
    from concourse.bass_utils import run_bass_kernel_spmd

    in_maps, t30, n_zero = _prepare(inputs, labels, emb_cq, label_cq,
                                    header_cq)

    if "prog" not in _PROG_CACHE:
        _PROG_CACHE["prog"] = _build_program()
    nc = _PROG_CACHE["prog"]

    res = run_bass_kernel_spmd(nc, in_maps, core_ids=list(range(N_CORES)))
    return _combine(res.results, t30, n_zero)
